# revision 1
# baseline (speedup 1.0000x reference)
"""Trainium2 Bass kernel for nn_AttentionLayer (GNN message passing).

Math (per node n, K=64 neighbors, E=512):
  reference computes LN->Linear on x and y, attention logits via W2 over
  cat([x_rep, y_]), softmax over K, weighted sum of y_, gelu(x + a).

Key simplifications (exact):
  - softmax over k is invariant to per-n shifts => prob depends only on
    s[n,k] = LN(y[n,k]) . (W1 @ w2y)  (x path, b1, b2 cancel entirely)
  - with g = gamma * (W1 @ w2y):  s = (d - m*S) / sigma  (+const),
    where d = y.g (raw dot), m = mean(y), sigma = std(y), S = sum(g)
  - a = (sum_k p_k LN(y_k)) @ W1 + b1
      = ((sum_k q_k y_k) - (sum_k q_k m_k)) @ (diag(gamma) W1) + beta@W1 + b1
    with q_k = p_k / sigma_k  => aggregate RAW y rows with TensorE matmuls,
    apply W1 once per node AFTER aggregation (64x fewer matmul FLOPs).

Layout: rows (n,k) on partitions (128 rows = 2 nodes x 64 neighbors per
tile), E on free dim. Per-row stats:
  - sum(y): free via accum_out of the f32->bf16 cast (tensor_scalar, 2x)
  - sum(y^2): ScalarE Square + accum_out
  - d = y.g: split between DVE tensor_tensor_reduce (1x) and
    [DVE bf16 tensor_tensor mult (2x) + ScalarE Copy + accum_out]
Aggregation: per 2-node tile, lhsT = block-diag q [128,2], rhs = y_bf16
[128,512] -> PSUM [2,512]; second matmul rhs=[m|sigma] gives r=sum q*m and
sumexp=sum q*sigma = sum exp(s) for free.

Sharding: data-parallel over B*L across 8 cores, params replicated.
"""

import os
import numpy as np
import ml_dtypes
from contextlib import ExitStack

import concourse.bass as bass
import concourse.mybir as mybir
import concourse.tile as tile
from concourse.bass_utils import run_bass_kernel_spmd
from concourse.masks import make_identity

F32 = mybir.dt.float32
BF16 = mybir.dt.bfloat16
AL = mybir.AluOpType
AF = mybir.ActivationFunctionType

B, L, K, E = 32, 64, 64, 512
NCORES = 8
N = B * L // NCORES          # 256 nodes per core
R = N * K                    # 16384 y-rows per core
P = 128                      # partitions
T = R // P                   # 128 tiles of [128, E] per core
BT = 32                      # tiles per block
NB = T // BT                 # blocks per core
EPS = 1e-5
INV_E = 1.0 / E

# fraction of tiles whose d-reduction runs on DVE via tensor_tensor_reduce
# (rest: DVE bf16 mult at 2x + ScalarE Copy+accum reduce). Tune for balance.
TTR_FRAC = float(os.environ.get("KERNEL_TTR_FRAC", "0.5"))


def split_waits(nc):
    """Workaround for this walrus build: most instruction structs encode only
    one sync-wait command, but Tile emits up to ~3 per instruction. Hoist all
    but the last wait onto same-engine NoOps spliced immediately before the
    instruction — engine queues are in-order, so waits executed by the NoOp
    are equivalent to waits on the instruction itself."""
    n_split = 0
    for f in nc.m.functions:
        for bb in f.blocks:
            insts = list(bb.instructions)
            out = []
            for inst in insts:
                si = inst.sync_info
                if si is not None and len(si.on_wait) > 1:
                    waits = list(si.on_wait)
                    for k, w in enumerate(waits[:-1]):
                        nop = mybir.InstNoOp(
                            name=f"{inst.name}-ws{k}", ins=[], outs=[])
                        nop.engine = inst.engine
                        nop.sync_info = mybir.SyncInfo(on_wait=[w],
                                                       on_update=[])
                        out.append(nop)
                        n_split += 1
                    inst.sync_info = mybir.SyncInfo(
                        on_wait=[waits[-1]], on_update=list(si.on_update))
                out.append(inst)
            bb.instructions = out
    return n_split


def build(ttr_frac=TTR_FRAC):
    nc = bass.Bass(trn_type="TRN2")

    x_d = nc.dram_tensor("x", [N, E], F32, kind="ExternalInput")
    yb_d = nc.dram_tensor("yb", [R, E], BF16, kind="ExternalInput")
    og_d = nc.dram_tensor("og", [4, P, 2], BF16, kind="ExternalInput")
    w1g_d = nc.dram_tensor("w1g", [4, P, E], F32, kind="ExternalInput")
    bb_d = nc.dram_tensor("bb", [1, E], F32, kind="ExternalInput")
    sS_d = nc.dram_tensor("sS", [P, 1], F32, kind="ExternalInput")
    out_d = nc.dram_tensor("out", [N, E], F32, kind="ExternalOutput")

    SB = 4 * P               # rows per stats sub-block (4 tiles)
    NSB = R // SB            # 32 sub-blocks per core
    SB_PER_BLK = BT // 4     # 8 sub-blocks per block

    with tile.TileContext(nc) as tc, ExitStack() as ctx:
        singles = ctx.enter_context(tc.tile_pool(name="singles", bufs=1))
        ztp = ctx.enter_context(tc.tile_pool(name="ztp", bufs=8))
        z2p = ctx.enter_context(tc.tile_pool(name="z2p", bufs=4))
        zpool = ctx.enter_context(tc.tile_pool(name="zpool", bufs=2))
        stats = ctx.enter_context(tc.tile_pool(name="stats", bufs=3))
        small = ctx.enter_context(tc.tile_pool(name="small", bufs=2))
        fpool = ctx.enter_context(tc.tile_pool(name="fpool", bufs=2))
        psS = ctx.enter_context(tc.tile_pool(name="psS", bufs=2, space="PSUM"))
        psA = ctx.enter_context(tc.tile_pool(name="psA", bufs=2, space="PSUM"))
        psR = ctx.enter_context(tc.tile_pool(name="psR", bufs=1, space="PSUM"))
        psT = ctx.enter_context(tc.tile_pool(name="psT", bufs=1, space="PSUM"))

        # constants
        og_t = singles.tile([P, 4, 2], BF16)
        nc.sync.dma_start(out=og_t, in_=og_d[:, :, :].rearrange("c p k -> p c k"))
        w1g_t = singles.tile([P, 4, E], F32)
        nc.sync.dma_start(out=w1g_t, in_=w1g_d[:, :, :].rearrange("c p e -> p c e"))
        bb_t = singles.tile([1, E], F32)
        nc.sync.dma_start(out=bb_t, in_=bb_d[:, :])
        sS_t = singles.tile([P, 1], F32)
        nc.sync.dma_start(out=sS_t, in_=sS_d[:, :])
        ones_row = singles.tile([1, P], F32)
        nc.vector.memset(ones_row, 1.0)
        ident = singles.tile([P, P], F32)
        make_identity(nc, ident)

        # per-row stats [sum z | sum z*g | sum z^2] bounced through a DRAM
        # scratch (PSUM->DRAM, then DRAM->SBUF in [row%128, tile] layout;
        # direct SBUF->SBUF partition-scatter APs don't balance)
        std_d = nc.dram_tensor("stats_scratch", [3, R], F32)

        # persistent PSUM accumulation targets (one per 128-node chunk)
        agg_ps = [psA.tile([P, E], F32, name=f"agg{i}") for i in range(2)]
        rs_all = psR.tile([P, 4], F32, name="rs_all")
        rs_ps = [rs_all[:, 0:2], rs_all[:, 2:4]]

        # full-width block-diag weight buffers: matmul out base partition must
        # be 0/32/64, so each tile's matmul writes all 128 psum rows with
        # lhsT zero outside the tile's 2 columns and accumulates (start only
        # on the chunk's first tile). Column for tile tg, half h:
        # 2*(tg%64)+h -> flat col 130*t + (blk%2)*64 + h. Two parity buffers,
        # zeroed once; per block only the q columns are rewritten.
        qfull = [singles.tile([P, BT * P], BF16, name=f"qfull{i}")
                 for i in range(2)]
        nc.gpsimd.memset(qfull[0], 0.0)
        nc.gpsimd.memset(qfull[1], 0.0)

        for blk in range(NB):
            zbf = zpool.tile([P, BT * E], BF16, tag="zbf")

            # ---- phase A: stats via TensorE on xbar-transposed loads ----
            # Per 512-row sub-block: 4 transposed tiles [128 e-chunk, 512
            # rows]; [1|g] matvec accumulates [sum z; sum z*g] into PSUM rows
            # 0:2, squared (ScalarE) matvec accumulates sum z^2 into row 32.
            for s in range(SB_PER_BLK):
                sbg = blk * SB_PER_BLK + s
                r0 = sbg * SB
                zts = []
                for c in range(4):
                    zt = ztp.tile([P, SB], BF16, tag="zt")
                    nc.sync.dma_start_transpose(
                        out=zt, in_=yb_d[r0:r0 + SB, c * P:(c + 1) * P])
                    zts.append(zt)
                st_ps = psS.tile([P, SB], F32, tag="st_ps")
                for c in range(4):
                    nc.tensor.matmul(
                        st_ps[0:2, :], og_t[:, c, :], zts[c],
                        start=(c == 0), stop=(c == 3))
                for c in range(4):
                    z2 = z2p.tile([P, SB], BF16, tag="z2")
                    nc.scalar.activation(out=z2, in_=zts[c], func=AF.Square)
                    nc.tensor.matmul(
                        st_ps[32:33, :], og_t[:, c, 0:1], z2,
                        start=(c == 0), stop=(c == 3))
                stcp = z2p.tile([P, SB], F32, tag="stcp")
                nc.vector.tensor_scalar(
                    out=stcp[0:33, :], in0=st_ps[0:33, :],
                    scalar1=1.0, scalar2=None, op0=AL.mult)
                nc.sync.dma_start(out=std_d[0:2, r0:r0 + SB],
                                  in_=stcp[0:2, :])
                nc.sync.dma_start(out=std_d[2:3, r0:r0 + SB],
                                  in_=stcp[32:33, :])

            # normal-layout bf16 y for the aggregation matmuls
            for c in range(SB_PER_BLK):
                r0 = (blk * BT + c * 4) * P
                nc.sync.dma_start(
                    out=zbf[:, c * 4 * E:(c + 1) * 4 * E].rearrange(
                        "p (c e) -> p c e", e=E),
                    in_=yb_d[r0:r0 + 4 * P, :].rearrange("(c p) e -> p c e", p=P),
                )

            # re-layout the block's stats to [row%128, tile]
            sumz = stats.tile([P, BT], F32, tag="sumz")
            dd = stats.tile([P, BT], F32, tag="dd")
            sumsq = stats.tile([P, BT], F32, tag="sumsq")
            b0 = blk * BT * P
            for st_row, dst in ((0, sumz), (1, dd), (2, sumsq)):
                nc.sync.dma_start(
                    out=dst,
                    in_=std_d[st_row, b0:b0 + BT * P].rearrange(
                        "(t p) -> p t", p=P),
                )

            # ---- phase B: batched per-row smalls -> q, [m|sigma] ----
            m_f = small.tile([P, BT], F32, tag="m")
            nc.vector.tensor_scalar(
                out=m_f, in0=sumz, scalar1=INV_E, scalar2=None, op0=AL.mult)
            ve = small.tile([P, BT], F32, tag="ve")
            nc.vector.tensor_scalar(
                out=ve, in0=sumsq, scalar1=INV_E, scalar2=EPS,
                op0=AL.mult, op1=AL.add)
            m2 = small.tile([P, BT], F32, tag="m2")
            nc.vector.tensor_mul(out=m2, in0=m_f, in1=m_f)
            nc.vector.tensor_sub(out=ve, in0=ve, in1=m2)
            sig = small.tile([P, BT], F32, tag="sig")
            nc.scalar.activation(out=sig, in_=ve, func=AF.Sqrt)
            isig = small.tile([P, BT], F32, tag="isig")
            nc.vector.reciprocal(out=isig, in_=sig)
            # s = (d - m*S) * isig
            ms = small.tile([P, BT], F32, tag="ms")
            nc.vector.tensor_scalar(
                out=ms, in0=m_f, scalar1=sS_t, scalar2=None, op0=AL.mult)
            nc.vector.tensor_sub(out=ms, in0=dd, in1=ms)
            nc.vector.tensor_mul(out=ms, in0=ms, in1=isig)
            exps = small.tile([P, BT], BF16, tag="exps")
            nc.scalar.activation(out=exps, in_=ms, func=AF.Exp)
            isig_bf = small.tile([P, BT], BF16, tag="isigbf")
            nc.vector.tensor_scalar(
                out=isig_bf, in0=isig, scalar1=1.0, scalar2=None, op0=AL.mult)
            q_bf = small.tile([P, BT], BF16, tag="qbf")
            nc.vector.tensor_mul(out=q_bf, in0=exps, in1=isig_bf)

            # write q into the parity buffer's block-diag columns
            qf = qfull[blk % 2]
            base = (blk % 2) * 64
            nc.vector.tensor_scalar(
                out=qf[0:64, base::130], in0=q_bf[0:64, :],
                scalar1=1.0, scalar2=None, op0=AL.mult)
            nc.vector.tensor_scalar(
                out=qf[64:128, base + 1::130], in0=q_bf[64:128, :],
                scalar1=1.0, scalar2=None, op0=AL.mult)
            # msig[:,2t] = m, msig[:,2t+1] = sigma (bf16)
            msig = small.tile([P, 2 * BT], BF16, tag="msig")
            mv = msig.rearrange("p (t two) -> p t two", two=2)
            nc.vector.tensor_scalar(
                out=mv[:, :, 0:1],
                in0=m_f.rearrange("p (t one) -> p t one", one=1),
                scalar1=1.0, scalar2=None, op0=AL.mult)
            nc.vector.tensor_scalar(
                out=mv[:, :, 1:2],
                in0=sig.rearrange("p (t one) -> p t one", one=1),
                scalar1=1.0, scalar2=None, op0=AL.mult)

            # ---- phase C: aggregation matmuls (2 nodes per tile) ----
            for t in range(BT):
                tg = blk * BT + t
                nck = tg // 64
                first = (tg % 64) == 0
                last = (tg % 64) == 63
                lhsT = qf[:, t * P:(t + 1) * P]
                nc.tensor.matmul(
                    agg_ps[nck], lhsT,
                    zbf[:, t * E:(t + 1) * E], start=first, stop=last)
                nc.tensor.matmul(
                    rs_ps[nck], lhsT,
                    msig[:, 2 * t:2 * t + 2], start=first, stop=last)

        # ---- final: normalize, transpose, W1g matmul, +bb, +x, gelu ----
        aggT = singles.tile([P, 4 * N], F32)  # [e_chunk(4) x nodes(256)]
        for ncx in range(2):
            rs_sb = fpool.tile([P, 2], F32, tag="rs")
            nc.vector.tensor_scalar(
                out=rs_sb, in0=rs_ps[ncx], scalar1=1.0, scalar2=None,
                op0=AL.mult)
            rinv = fpool.tile([P, 1], F32, tag="rinv")
            nc.vector.reciprocal(out=rinv, in_=rs_sb[:, 1:2])
            aggn = fpool.tile([P, E], F32, tag="aggn")
            nc.vector.tensor_scalar(
                out=aggn, in0=agg_ps[ncx], scalar1=rs_sb[:, 0:1],
                scalar2=rinv, op0=AL.subtract, op1=AL.mult)
            for c in range(4):
                tp = psT.tile([P, P], F32, tag="tp")
                nc.tensor.transpose(tp, aggn[:, c * P:(c + 1) * P], ident)
                nc.vector.tensor_scalar(
                    out=aggT[:, c * N + ncx * P: c * N + (ncx + 1) * P],
                    in0=tp, scalar1=1.0, scalar2=None, op0=AL.mult)

        for ncx in range(2):
            fin = psS.tile([P, E], F32, tag="st_ps", name="fin")
            for c in range(4):
                nc.tensor.matmul(
                    fin, aggT[:, c * N + ncx * P: c * N + (ncx + 1) * P],
                    w1g_t[:, c, :], start=(c == 0), stop=False)
            nc.tensor.matmul(
                fin, ones_row[0:1, :], bb_t[0:1, :], start=False, stop=True)
            xin = fpool.tile([P, E], F32, tag="xin")
            nc.sync.dma_start(out=xin, in_=x_d[ncx * P:(ncx + 1) * P, :])
            pre = fpool.tile([P, E], F32, tag="pre")
            nc.vector.tensor_add(out=pre, in0=fin, in1=xin)
            outt = fpool.tile([P, E], F32, tag="outt")
            nc.scalar.activation(out=outt, in_=pre, func=AF.Gelu_apprx_tanh)
            nc.sync.dma_start(out=out_d[ncx * P:(ncx + 1) * P, :], in_=outt)

    split_waits(nc)
    return nc


_NC_CACHE = {}


def make_in_maps(x, y, ln_gamma, ln_beta, W1, b1, W2, b2):
    x = np.asarray(x, np.float32)
    y = np.asarray(y, np.float32)
    ln_gamma = np.asarray(ln_gamma, np.float32)
    ln_beta = np.asarray(ln_beta, np.float32)
    W1 = np.asarray(W1, np.float32)
    b1 = np.asarray(b1, np.float32)
    W2 = np.asarray(W2, np.float32)

    # host-side precomputation (cheap, E-sized)
    w2y = W2[E:]
    v = W1 @ w2y                          # [E]
    g = (ln_gamma * v).astype(np.float32)  # [E]
    S = float(g.sum())
    w1g = (ln_gamma[:, None] * W1).astype(np.float32)      # [E, E]
    bb = (ln_beta @ W1 + b1).astype(np.float32)            # [E]

    g_bf = g.astype(ml_dtypes.bfloat16)
    og = np.empty((4, P, 2), ml_dtypes.bfloat16)
    og[:, :, 0] = np.float32(1.0)
    og[:, :, 1] = g_bf.reshape(4, P)
    w1g_c = w1g.reshape(4, P, E).copy()
    bb_r = bb.reshape(1, E)
    sS = np.full((P, 1), S, np.float32)

    y_bf = y.reshape(B * L, K, E).astype(ml_dtypes.bfloat16)
    x_f = x.reshape(B * L, E)
    in_maps = []
    for i in range(NCORES):
        in_maps.append({
            "x": np.ascontiguousarray(x_f[i * N:(i + 1) * N]),
            "yb": np.ascontiguousarray(
                y_bf[i * N:(i + 1) * N].reshape(R, E)),
            "og": og,
            "w1g": w1g_c,
            "bb": bb_r,
            "sS": sS,
        })
    return in_maps


def kernel(x, y, ln_gamma, ln_beta, W1, b1, W2, b2, select_indegree_num=None,
           **kw):
    in_maps = make_in_maps(x, y, ln_gamma, ln_beta, W1, b1, W2, b2)
    if "nc" not in _NC_CACHE:
        _NC_CACHE["nc"] = build()
    nc = _NC_CACHE["nc"]

    res = run_bass_kernel_spmd(nc, in_maps, core_ids=list(range(NCORES)),
                               trace=bool(int(os.environ.get("KERNEL_TRACE", "0"))))
    _NC_CACHE["last_result"] = res
    out = np.concatenate([r["out"] for r in res.results], axis=0)
    return out.reshape(B, L, E)



# revision 17
# speedup vs baseline: 2.6619x; 2.6619x over previous
"""Trainium2 Bass kernel for nn_AttentionLayer (GNN message passing).

Math (per node n, K=64 neighbors, E=512), derived from the reference:
  - softmax over k is invariant to per-n shifts => prob depends only on
    s[n,k] = (d - m*S)/sigma, where d = y.g (g = gamma*(W1@w2y)), m/sigma
    the per-row LN stats, S = sum(g).  x path, b1, b2 cancel entirely.
  - a = ((sum_k q_k y_k) - (sum_k q_k m_k)) @ (diag(gamma) W1) + beta@W1 + b1
    with q_k = exp(s_k)/sigma_k / sumexp, sumexp = sum_k exp(s_k) = sum q~ sigma.

Device layout strategy (the baseline was DMA-packet-bound: xbar-transpose
DMAs at 256 B/packet + a DRAM stats bounce at 4 B/packet => ~167 GB/s
aggregate and 465 us).  Here ALL DMAs are large with per-partition
contiguous DRAM, permuted host-side:
  - yn8 [128, T*E] fp8:  [p, t*E+e] = y[128*t+p, e]  (normal, row-major tiles)
    loaded with a casting SWDGE DMA into bf16 SBUF chunks (halves HBM traffic)
  - yt8 [128, 4, R] fp8: [p, c, r] = y[r, 128*c+p]   (host-side transpose)
    feeds TensorE [1|g] matvecs => per-row {sum z, d} with NO on-chip transpose
  - z^2 stats via ScalarE Square+accum_out / DVE mult+accum split, from the
    bf16 normal-layout chunks.
  - stats [2, 512] PSUM blocks are relayouted to [row%128, tile] with thin
    PE transposes (no DRAM bounce).
  - sigma via exp(+-0.5*ln(var+eps)): Square/Ln/Exp share ONE activation
    table set (natural_log_exp_and_others) -> no table thrashing; one switch
    for the final Gelu.

Sharding: data-parallel over B*L across 8 cores, params replicated.
"""

import os
import numpy as np
import ml_dtypes
from contextlib import ExitStack

import concourse.bass as bass
import concourse.mybir as mybir
import concourse.tile as tile
from concourse.bass_utils import run_bass_kernel_spmd
from concourse.masks import make_identity

F32 = mybir.dt.float32
BF16 = mybir.dt.bfloat16
FP8 = mybir.dt.float8e4
AL = mybir.AluOpType
AF = mybir.ActivationFunctionType

B, L, K, E = 32, 64, 64, 512
NCORES = 8
N = B * L // NCORES          # 256 nodes per core
R = N * K                    # 16384 y-rows per core
P = 128                      # partitions
T = R // P                   # 128 tiles of [128, E] per core
CH = 16                      # tiles per chunk
NCH = T // CH                # 8 chunks
CHR = CH * P                 # 2048 rows per chunk
EPS = 1e-5
INV_E = 1.0 / E

# knobs
USE_CAST_DMA = bool(int(os.environ.get("KERNEL_CAST_DMA", "1")))
Z2_DVE = int(os.environ.get("KERNEL_Z2_DVE", "9"))   # of 16 tiles/chunk on DVE
CP_SCALAR = int(os.environ.get("KERNEL_CP_SCALAR", "2"))  # of 4 grp copies on ScalarE
SIM_NO_GELU = bool(int(os.environ.get("KERNEL_SIM_NO_GELU", "0")))  # CoreSim lacks Gelu


def split_waits(nc):
    """Workaround for this walrus build: most instruction structs encode only
    one sync-wait command, but Tile emits up to ~3 per instruction. Hoist all
    but the last wait onto same-engine NoOps spliced immediately before the
    instruction."""
    n_split = 0
    for f in nc.m.functions:
        for bb in f.blocks:
            insts = list(bb.instructions)
            out = []
            for inst in insts:
                si = inst.sync_info
                if si is not None and len(si.on_wait) > 1:
                    waits = list(si.on_wait)
                    for k, w in enumerate(waits[:-1]):
                        nop = mybir.InstNoOp(
                            name=f"{inst.name}-ws{k}", ins=[], outs=[])
                        nop.engine = inst.engine
                        nop.sync_info = mybir.SyncInfo(on_wait=[w],
                                                       on_update=[])
                        out.append(nop)
                        n_split += 1
                    inst.sync_info = mybir.SyncInfo(
                        on_wait=[waits[-1]], on_update=list(si.on_update))
                out.append(inst)
            bb.instructions = out
    return n_split


def build():
    nc = bass.Bass(trn_type="TRN2")

    yn_dt = FP8 if USE_CAST_DMA else BF16
    yn_d = nc.dram_tensor("yn8", [P, T * E], yn_dt, kind="ExternalInput")
    yt_d = nc.dram_tensor("yt8", [P, 4, R], FP8, kind="ExternalInput")
    x_d = nc.dram_tensor("x", [P, 2, E], F32, kind="ExternalInput")
    og_d = nc.dram_tensor("og", [P, 4, 2], BF16, kind="ExternalInput")
    w1g_d = nc.dram_tensor("w1g", [P, 4, E], F32, kind="ExternalInput")
    bb_d = nc.dram_tensor("bb", [1, E], F32, kind="ExternalInput")
    sS_d = nc.dram_tensor("sS", [P, 1], F32, kind="ExternalInput")
    out_d = nc.dram_tensor("out", [P, 2, E], F32, kind="ExternalOutput")

    with tile.TileContext(nc) as tc, ExitStack() as ctx:
        singles = ctx.enter_context(tc.tile_pool(name="singles", bufs=1))
        ynp = ctx.enter_context(tc.tile_pool(name="ynp", bufs=3))
        ytp = ctx.enter_context(tc.tile_pool(name="ytp", bufs=2))
        stp = ctx.enter_context(tc.tile_pool(name="stp", bufs=8))
        stats = ctx.enter_context(tc.tile_pool(name="stats", bufs=3))
        junkp = ctx.enter_context(tc.tile_pool(name="junkp", bufs=4))
        small = ctx.enter_context(tc.tile_pool(name="small", bufs=3))
        fpool = ctx.enter_context(tc.tile_pool(name="fpool", bufs=2))
        psS = ctx.enter_context(tc.tile_pool(name="psS", bufs=2, space="PSUM"))
        psA = ctx.enter_context(tc.tile_pool(name="psA", bufs=1, space="PSUM"))
        psR = ctx.enter_context(tc.tile_pool(name="psR", bufs=1, space="PSUM"))
        psT = ctx.enter_context(tc.tile_pool(name="psT", bufs=2, space="PSUM"))

        # chunk-load stage, defined early so chunks 0/1 can be prefetched
        # ahead of the parameter loads (nothing blocks on params for a while)
        st8 = {}

        def stage_load(ch):
            yn = ynp.tile([P, CH * E], BF16, tag="yn")
            src = yn_d[:, ch * CH * E:(ch + 1) * CH * E]
            if USE_CAST_DMA:
                nc.gpsimd.dma_start(out=yn, in_=src)
            else:
                nc.sync.dma_start(out=yn, in_=src)
            yt = ytp.tile([P, 4, CHR], FP8, tag="yt")
            nc.sync.dma_start(out=yt, in_=yt_d[:, :, ch * CHR:(ch + 1) * CHR])
            st8[ch] = {"yn": yn, "yt": yt}

        stage_load(0)
        stage_load(1)

        # ---- constants needed by the main loop ----
        og_t = singles.tile([P, 4, 2], BF16)
        nc.sync.dma_start(out=og_t, in_=og_d[:, :, :])
        sS_t = singles.tile([P, 1], F32)
        nc.sync.dma_start(out=sS_t, in_=sS_d[:, :])
        ones_row = singles.tile([1, P], F32)
        nc.vector.memset(ones_row, 1.0)
        ident = singles.tile([P, P], F32)
        make_identity(nc, ident)
        # final-phase params (loaded later, mid-loop, when SP has slack)
        w1g_t = singles.tile([P, 4, E], F32)
        bb_t = singles.tile([1, E], F32)
        x_t = singles.tile([P, 2, E], F32)

        def load_final_params():
            nc.sync.dma_start(out=w1g_t, in_=w1g_d[:, :, :])
            nc.sync.dma_start(out=bb_t, in_=bb_d[:, :])
            nc.sync.dma_start(out=x_t, in_=x_d[:, :, :])

        # block-diag aggregation weights. Each buffer owns a FIXED disjoint
        # column window (win j covers local cols 32j..32j+31); anything else
        # stays zero forever, so a tile-slice lhsT never picks up stale q
        # from other chunks. Buffer j is reused by chunks j and j+4 (same
        # window; WAR tracked by Tile).
        qf = [singles.tile([P, CH * P], BF16, name=f"qf{i}") for i in range(4)]
        for i in range(4):
            nc.gpsimd.memset(qf[i], 0.0)

        # persistent PSUM accumulation targets (one per 128-node chunk)
        agg_ps = [psA.tile([P, E], F32, name=f"agg{i}") for i in range(2)]
        rs_ps = [psR.tile([P, 2], F32, name=f"rs{i}") for i in range(2)]

        # Software-pipelined emission: per iteration we emit
        #   Pf(ch+1): DMA prefetch          (issued 1 iter ahead)
        #   B(ch-1):  transposes/smalls/q   (consumes stats of prev chunk)
        #   A(ch):    stats MMs, z^2        (consumes prefetched loads)
        #   C(ch-2):  aggregation MMs       (consumes q of 2 chunks back)
        # so each engine's in-order queue only ever waits on work emitted a
        # full iteration earlier -> no head-of-line stalls.
        def stage_a(ch):
            s = st8[ch]
            yn, yt = s["yn"], s["yt"]
            # TensorE [1|g] matvec over transposed fp8 -> {sum z, d} per row
            sbs = []
            for g in range(4):
                st_ps = psS.tile([2, 512], F32, tag="st")
                for c in range(4):
                    nc.tensor.matmul(
                        st_ps, og_t[:, c, :], yt[:, c, g * 512:(g + 1) * 512],
                        start=(c == 0), stop=(c == 3))
                st_sb = stp.tile([2, 512], F32, tag="stsb")
                if g % 4 < CP_SCALAR:
                    nc.scalar.activation(out=st_sb, in_=st_ps, func=AF.Copy)
                else:
                    nc.vector.tensor_scalar(
                        out=st_sb, in0=st_ps, scalar1=1.0, scalar2=None,
                        op0=AL.mult)
                sbs.append(st_sb)
            # z^2 row sums: ScalarE Square+accum / DVE mult+reduce split
            ssq = stats.tile([P, CH], F32, tag="ssq")
            for t in range(CH):
                zt = yn[:, t * E:(t + 1) * E]
                if t < Z2_DVE:
                    prod = junkp.tile([P, E], BF16, tag="jv")
                    nc.vector.tensor_mul(out=prod, in0=zt, in1=zt)
                    jv2 = junkp.tile([P, E], BF16, tag="jv2")
                    nc.vector.tensor_scalar(
                        out=jv2, in0=prod, scalar1=1.0, scalar2=None,
                        op0=AL.mult, op1=AL.add, accum_out=ssq[:, t:t + 1])
                else:
                    js = junkp.tile([P, E], BF16, tag="js")
                    nc.scalar.activation(
                        out=js, in_=zt, func=AF.Square,
                        accum_out=ssq[:, t:t + 1])
            s["sbs"] = sbs
            s["ssq"] = ssq

        def stage_b(ch):
            s = st8[ch]
            sbs, ssq = s["sbs"], s["ssq"]
            stT_ps = psT.tile([P, 2 * CH], F32, tag="stT")
            for g in range(4):
                for t4 in range(4):
                    t = g * 4 + t4
                    nc.tensor.transpose(
                        stT_ps[:, 2 * t:2 * t + 2],
                        sbs[g][:, t4 * P:(t4 + 1) * P], ident[0:2, 0:2])
            szd = stats.tile([P, 2 * CH], F32, tag="szd")
            nc.vector.tensor_scalar(
                out=szd, in0=stT_ps, scalar1=1.0, scalar2=None, op0=AL.mult)

            # ---- smalls: m, var, sigma^{+-1} via exp/ln, logits, q ----
            m_f = small.tile([P, CH], F32, tag="m")
            nc.vector.tensor_scalar(
                out=m_f, in0=szd[:, 0:2 * CH:2], scalar1=INV_E, scalar2=None,
                op0=AL.mult)
            ve = small.tile([P, CH], F32, tag="ve")
            nc.vector.tensor_scalar(
                out=ve, in0=ssq, scalar1=INV_E, scalar2=EPS,
                op0=AL.mult, op1=AL.add)
            m2 = small.tile([P, CH], F32, tag="m2")
            nc.vector.tensor_mul(out=m2, in0=m_f, in1=m_f)
            nc.vector.tensor_sub(out=ve, in0=ve, in1=m2)
            lnv = small.tile([P, CH], F32, tag="lnv")
            nc.scalar.activation(out=lnv, in_=ve, func=AF.Ln)
            isig = small.tile([P, CH], F32, tag="isig")
            nc.scalar.activation(out=isig, in_=lnv, func=AF.Exp, scale=-0.5)
            sig_bf = small.tile([P, CH], BF16, tag="sigbf")
            nc.scalar.activation(out=sig_bf, in_=lnv, func=AF.Exp, scale=0.5)
            # s = (d - m*S) * isig
            ms = small.tile([P, CH], F32, tag="ms")
            nc.vector.tensor_scalar(
                out=ms, in0=m_f, scalar1=sS_t, scalar2=None, op0=AL.mult)
            nc.vector.tensor_sub(out=ms, in0=szd[:, 1:2 * CH:2], in1=ms)
            nc.vector.tensor_mul(out=ms, in0=ms, in1=isig)
            exps = small.tile([P, CH], BF16, tag="exps")
            nc.scalar.activation(out=exps, in_=ms, func=AF.Exp)
            isig_bf = small.tile([P, CH], BF16, tag="isigbf")
            nc.vector.tensor_scalar(
                out=isig_bf, in0=isig, scalar1=1.0, scalar2=None, op0=AL.mult)
            q_bf = small.tile([P, CH], BF16, tag="qbf")
            nc.vector.tensor_mul(out=q_bf, in0=exps, in1=isig_bf)

            # write q into qf[ch%4]'s fixed block-diag column window:
            # col(t, h) = 130*t + 32*(ch%4) + h
            qfb = qf[ch % 4]
            base = 32 * (ch % 4)
            nc.vector.tensor_scalar(
                out=qfb[0:64, base::130], in0=q_bf[0:64, :],
                scalar1=1.0, scalar2=None, op0=AL.mult)
            nc.vector.tensor_scalar(
                out=qfb[64:128, base + 1::130], in0=q_bf[64:128, :],
                scalar1=1.0, scalar2=None, op0=AL.mult)
            # msig[:,2t] = m, msig[:,2t+1] = sigma (bf16)
            msig = small.tile([P, 2 * CH], BF16, tag="msig")
            mv = msig.rearrange("p (t two) -> p t two", two=2)
            nc.vector.tensor_scalar(
                out=mv[:, :, 0:1],
                in0=m_f.rearrange("p (t one) -> p t one", one=1),
                scalar1=1.0, scalar2=None, op0=AL.mult)
            nc.vector.tensor_scalar(
                out=mv[:, :, 1:2],
                in0=sig_bf.rearrange("p (t one) -> p t one", one=1),
                scalar1=1.0, scalar2=None, op0=AL.mult)
            s["msig"] = msig

        def stage_c(ch):
            s = st8[ch]
            yn, msig = s["yn"], s["msig"]
            qfb = qf[ch % 4]
            nck = ch // 4
            for t in range(CH):
                lhsT = qfb[:, t * P:(t + 1) * P]
                first = (ch % 4 == 0) and t == 0
                last = (ch % 4 == 3) and t == CH - 1
                nc.tensor.matmul(
                    agg_ps[nck], lhsT, yn[:, t * E:(t + 1) * E],
                    start=first, stop=last)
                nc.tensor.matmul(
                    rs_ps[nck], lhsT, msig[:, 2 * t:2 * t + 2],
                    start=first, stop=last)
            del st8[ch]

        # ---- final phase (split): head = normalize/transpose/W1g matmul/+x
        # (no ScalarE, so group 0's head can run mid-loop); gelu + store at
        # the very end (single activation-table switch).
        aggT = singles.tile([P, 4 * N], F32)  # [e_chunk(4) x nodes(256)]

        def final_head(ncx):
            rs_sb = fpool.tile([P, 2], F32, tag="rs")
            nc.vector.tensor_scalar(
                out=rs_sb, in0=rs_ps[ncx], scalar1=1.0, scalar2=None,
                op0=AL.mult)
            rinv = fpool.tile([P, 1], F32, tag="rinv")
            nc.vector.reciprocal(out=rinv, in_=rs_sb[:, 1:2])
            aggn = fpool.tile([P, E], F32, tag="aggn")
            nc.vector.tensor_scalar(
                out=aggn, in0=agg_ps[ncx], scalar1=rs_sb[:, 0:1],
                scalar2=rinv, op0=AL.subtract, op1=AL.mult)
            for c in range(4):
                tp = psT.tile([P, P], F32, tag="stT")
                nc.tensor.transpose(tp, aggn[:, c * P:(c + 1) * P], ident)
                nc.vector.tensor_scalar(
                    out=aggT[:, c * N + ncx * P: c * N + (ncx + 1) * P],
                    in0=tp, scalar1=1.0, scalar2=None, op0=AL.mult)
            fin = agg_ps[ncx]  # dead after aggn copy -> reuse the PSUM bank
            for c in range(4):
                nc.tensor.matmul(
                    fin, aggT[:, c * N + ncx * P: c * N + (ncx + 1) * P],
                    w1g_t[:, c, :], start=(c == 0), stop=False)
            nc.tensor.matmul(
                fin, ones_row[0:1, :], bb_t[0:1, :], start=False, stop=True)
            pre = fpool.tile([P, E], F32, tag=f"pre{ncx}")
            nc.vector.tensor_add(out=pre, in0=fin, in1=x_t[:, ncx, :])
            return pre

        pre_t = [None, None]
        for it in range(NCH + 2):
            if 1 <= it and it + 1 < NCH:
                stage_load(it + 1)
            if it == 3:
                load_final_params()
            if 1 <= it <= NCH:
                stage_b(it - 1)
            if it < NCH:
                stage_a(it)
            if it >= 2:
                stage_c(it - 2)
            if it == 6:
                pre_t[0] = final_head(0)
        pre_t[1] = final_head(1)

        gelu_fn = AF.Copy if SIM_NO_GELU else AF.Gelu_apprx_tanh
        for ncx in range(2):
            outt = fpool.tile([P, E], F32, tag="outt")
            nc.scalar.activation(out=outt, in_=pre_t[ncx], func=gelu_fn)
            nc.sync.dma_start(out=out_d[:, ncx, :], in_=outt)

    split_waits(nc)
    return nc


_NC_CACHE = {}


def make_in_maps(x, y, ln_gamma, ln_beta, W1, b1, W2, b2):
    x = np.asarray(x, np.float32)
    y = np.asarray(y, np.float32)
    ln_gamma = np.asarray(ln_gamma, np.float32)
    ln_beta = np.asarray(ln_beta, np.float32)
    W1 = np.asarray(W1, np.float32)
    b1 = np.asarray(b1, np.float32)
    W2 = np.asarray(W2, np.float32)

    # host-side precomputation (cheap, E-sized)
    w2y = W2[E:]
    v = W1 @ w2y                          # [E]
    g = (ln_gamma * v).astype(np.float32)  # [E]
    S = float(g.sum())
    w1g = (ln_gamma[:, None] * W1).astype(np.float32)      # [E, E]
    bb = (ln_beta @ W1 + b1).astype(np.float32)            # [E]

    og = np.empty((P, 4, 2), ml_dtypes.bfloat16)
    og[:, :, 0] = np.float32(1.0)
    og[:, :, 1] = g.reshape(4, P).T.astype(ml_dtypes.bfloat16)
    w1g_t = np.ascontiguousarray(
        w1g.reshape(4, P, E).transpose(1, 0, 2))           # [P, 4, E]
    bb_r = bb.reshape(1, E)
    sS = np.full((P, 1), S, np.float32)

    f8 = ml_dtypes.float8_e4m3fn
    y8 = y.reshape(B * L, K, E).astype(f8)                 # quantize once
    x_f = x.reshape(B * L, E)
    in_maps = []
    for i in range(NCORES):
        yc = y8[i * N:(i + 1) * N].reshape(R, E)           # [R, E] fp8
        # normal partition-major: [p, t*E+e] = yc[t*128+p, e]
        yn = np.ascontiguousarray(
            yc.reshape(T, P, E).transpose(1, 0, 2)).reshape(P, T * E)
        if not USE_CAST_DMA:
            yn = yn.astype(ml_dtypes.bfloat16)
        # host transpose: [p, c, r] = yc[r, c*128+p]
        yt = np.ascontiguousarray(yc.reshape(R, 4, P).transpose(2, 1, 0))
        xc = np.ascontiguousarray(
            x_f[i * N:(i + 1) * N].reshape(2, P, E).transpose(1, 0, 2))
        in_maps.append({
            "yn8": yn, "yt8": yt, "x": xc,
            "og": og, "w1g": w1g_t, "bb": bb_r, "sS": sS,
        })
    return in_maps


def kernel(x, y, ln_gamma, ln_beta, W1, b1, W2, b2, select_indegree_num=None,
           **kw):
    in_maps = make_in_maps(x, y, ln_gamma, ln_beta, W1, b1, W2, b2)
    if "nc" not in _NC_CACHE:
        _NC_CACHE["nc"] = build()
    nc = _NC_CACHE["nc"]

    res = run_bass_kernel_spmd(nc, in_maps, core_ids=list(range(NCORES)),
                               trace=bool(int(os.environ.get("KERNEL_TRACE", "0"))))
    _NC_CACHE["last_result"] = res
    # out [P, 2, E] node-major -> [N, E]
    out = np.concatenate(
        [np.asarray(r["out"]).transpose(1, 0, 2).reshape(N, E)
         for r in res.results], axis=0)
    return out.reshape(B, L, E)


# revision 21
# speedup vs baseline: 2.8837x; 1.0833x over previous
"""Trainium2 Bass kernel for nn_AttentionLayer (GNN message passing).

Math (per node n, K=64 neighbors, E=512), derived from the reference:
  - softmax over k is invariant to per-n shifts => prob depends only on
    s[n,k] = (d - m*S)/sigma, where d = y.g (g = gamma*(W1@w2y)), m/sigma
    the per-row LN stats, S = sum(g).  x path, b1, b2 cancel entirely.
  - a = ((sum_k q_k y_k) - (sum_k q_k m_k)) @ (diag(gamma) W1) + beta@W1 + b1
    with q_k = exp(s_k)/sigma_k / sumexp, sumexp = sum_k exp(s_k) = sum q~ sigma.

Device layout strategy (the baseline was DMA-packet-bound: xbar-transpose
DMAs at 256 B/packet + a DRAM stats bounce at 4 B/packet => ~167 GB/s
aggregate and 465 us).  Here ALL DMAs are large with per-partition
contiguous DRAM, permuted host-side:
  - yn8 [128, T*E] fp8:  [p, t*E+e] = y[128*t+p, e]  (normal, row-major tiles)
    loaded with a casting SWDGE DMA into bf16 SBUF chunks (halves HBM traffic)
  - yt8 [128, 4, R] fp8: [p, c, r] = y[r, 128*c+p]   (host-side transpose)
    feeds TensorE [1|g] matvecs => per-row {sum z, d} with NO on-chip transpose
  - z^2 stats via ScalarE Square+accum_out / DVE mult+accum split, from the
    bf16 normal-layout chunks.
  - stats [2, 512] PSUM blocks are relayouted to [row%128, tile] with thin
    PE transposes (no DRAM bounce).
  - sigma via exp(+-0.5*ln(var+eps)): Square/Ln/Exp share ONE activation
    table set (natural_log_exp_and_others) -> no table thrashing; one switch
    for the final Gelu.

Sharding: data-parallel over B*L across 8 cores, params replicated.
"""

import os
import numpy as np
import ml_dtypes
from contextlib import ExitStack

import concourse.bass as bass
import concourse.mybir as mybir
import concourse.tile as tile
from concourse.bass_utils import run_bass_kernel_spmd
from concourse.masks import make_identity

F32 = mybir.dt.float32
BF16 = mybir.dt.bfloat16
FP8 = mybir.dt.float8e4
AL = mybir.AluOpType
AF = mybir.ActivationFunctionType

B, L, K, E = 32, 64, 64, 512
NCORES = 8
N = B * L // NCORES          # 256 nodes per core
R = N * K                    # 16384 y-rows per core
P = 128                      # partitions
T = R // P                   # 128 tiles of [128, E] per core
CH = 16                      # tiles per chunk
NCH = T // CH                # 8 chunks
CHR = CH * P                 # 2048 rows per chunk
EPS = 1e-5
INV_E = 1.0 / E

# knobs
USE_CAST_DMA = bool(int(os.environ.get("KERNEL_CAST_DMA", "1")))
SQ_SCAL = int(os.environ.get("KERNEL_SQ_SCAL", "12"))  # z^2 tiles squared on ScalarE
CP_SCALAR = int(os.environ.get("KERNEL_CP_SCALAR", "4"))  # of 4 grp copies on ScalarE
SIM_NO_GELU = bool(int(os.environ.get("KERNEL_SIM_NO_GELU", "0")))  # CoreSim lacks Gelu


def split_waits(nc):
    """Workaround for this walrus build: most instruction structs encode only
    one sync-wait command, but Tile emits up to ~3 per instruction. Hoist all
    but the last wait onto same-engine NoOps spliced immediately before the
    instruction."""
    n_split = 0
    for f in nc.m.functions:
        for bb in f.blocks:
            insts = list(bb.instructions)
            out = []
            for inst in insts:
                si = inst.sync_info
                if si is not None and len(si.on_wait) > 1:
                    waits = list(si.on_wait)
                    for k, w in enumerate(waits[:-1]):
                        nop = mybir.InstNoOp(
                            name=f"{inst.name}-ws{k}", ins=[], outs=[])
                        nop.engine = inst.engine
                        nop.sync_info = mybir.SyncInfo(on_wait=[w],
                                                       on_update=[])
                        out.append(nop)
                        n_split += 1
                    inst.sync_info = mybir.SyncInfo(
                        on_wait=[waits[-1]], on_update=list(si.on_update))
                out.append(inst)
            bb.instructions = out
    return n_split


def build():
    nc = bass.Bass(trn_type="TRN2")

    yn_dt = FP8 if USE_CAST_DMA else BF16
    yn_d = nc.dram_tensor("yn8", [P, T * E], yn_dt, kind="ExternalInput")
    yt_d = nc.dram_tensor("yt8", [P, 4, R], FP8, kind="ExternalInput")
    x_d = nc.dram_tensor("x", [P, 2, E], F32, kind="ExternalInput")
    og_d = nc.dram_tensor("og", [P, 4, 2], BF16, kind="ExternalInput")
    w1g_d = nc.dram_tensor("w1g", [P, 4, E], F32, kind="ExternalInput")
    bb_d = nc.dram_tensor("bb", [1, E], F32, kind="ExternalInput")
    sS_d = nc.dram_tensor("sS", [P, 1], F32, kind="ExternalInput")
    out_d = nc.dram_tensor("out", [P, 2, E], F32, kind="ExternalOutput")

    with tile.TileContext(nc) as tc, ExitStack() as ctx:
        singles = ctx.enter_context(tc.tile_pool(name="singles", bufs=1))
        ynp = ctx.enter_context(tc.tile_pool(name="ynp", bufs=3))
        ytp = ctx.enter_context(tc.tile_pool(name="ytp", bufs=2))
        stp = ctx.enter_context(tc.tile_pool(name="stp", bufs=3))
        stats = ctx.enter_context(tc.tile_pool(name="stats", bufs=3))
        foldp = ctx.enter_context(tc.tile_pool(name="foldp", bufs=2))
        small = ctx.enter_context(tc.tile_pool(name="small", bufs=3))
        fpool = ctx.enter_context(tc.tile_pool(name="fpool", bufs=2))
        psS = ctx.enter_context(tc.tile_pool(name="psS", bufs=2, space="PSUM"))
        psA = ctx.enter_context(tc.tile_pool(name="psA", bufs=1, space="PSUM"))
        psR = ctx.enter_context(tc.tile_pool(name="psR", bufs=1, space="PSUM"))
        psT = ctx.enter_context(tc.tile_pool(name="psT", bufs=2, space="PSUM"))

        # chunk-load stage, defined early so chunks 0/1 can be prefetched
        # ahead of the parameter loads (nothing blocks on params for a while)
        st8 = {}

        def stage_load(ch):
            yn = ynp.tile([P, CH * E], BF16, tag="yn")
            src = yn_d[:, ch * CH * E:(ch + 1) * CH * E]
            if USE_CAST_DMA:
                nc.gpsimd.dma_start(out=yn, in_=src)
            else:
                nc.sync.dma_start(out=yn, in_=src)
            yt = ytp.tile([P, 4, CHR], FP8, tag="yt")
            nc.sync.dma_start(out=yt, in_=yt_d[:, :, ch * CHR:(ch + 1) * CHR])
            st8[ch] = {"yn": yn, "yt": yt}

        stage_load(0)
        stage_load(1)

        # ---- constants needed by the main loop ----
        og_t = singles.tile([P, 4, 2], BF16)
        nc.sync.dma_start(out=og_t, in_=og_d[:, :, :])
        sS_t = singles.tile([P, 1], F32)
        nc.sync.dma_start(out=sS_t, in_=sS_d[:, :])
        ones_row = singles.tile([1, P], F32)
        nc.vector.memset(ones_row, 1.0)
        ident = singles.tile([P, P], F32)
        make_identity(nc, ident)
        # final-phase params (loaded later, mid-loop, when SP has slack)
        w1g_t = singles.tile([P, 4, E], F32)
        bb_t = singles.tile([1, E], F32)
        x_t = singles.tile([P, 2, E], F32)

        def load_final_params():
            nc.sync.dma_start(out=w1g_t, in_=w1g_d[:, :, :])
            nc.sync.dma_start(out=bb_t, in_=bb_d[:, :])
            nc.sync.dma_start(out=x_t, in_=x_d[:, :, :])

        # block-diag aggregation weights. Each buffer owns a FIXED disjoint
        # column window (win j covers local cols 32j..32j+31); anything else
        # stays zero forever, so a tile-slice lhsT never picks up stale q
        # from other chunks. Buffer j is reused by chunks j and j+4 (same
        # window; WAR tracked by Tile).
        qf = [singles.tile([P, CH * P], BF16, name=f"qf{i}") for i in range(4)]
        for i in range(4):
            nc.gpsimd.memset(qf[i], 0.0)

        # persistent PSUM accumulation targets (one per 128-node chunk)
        agg_ps = [psA.tile([P, E], F32, name=f"agg{i}") for i in range(2)]
        rs_ps = [psR.tile([P, 2], F32, name=f"rs{i}") for i in range(2)]

        # Software-pipelined emission: per iteration we emit
        #   Pf(ch+1): DMA prefetch          (issued 1 iter ahead)
        #   B(ch-1):  transposes/smalls/q   (consumes stats of prev chunk)
        #   A(ch):    stats MMs, z^2        (consumes prefetched loads)
        #   C(ch-2):  aggregation MMs       (consumes q of 2 chunks back)
        # so each engine's in-order queue only ever waits on work emitted a
        # full iteration earlier -> no head-of-line stalls.
        def tree_reduce(src_bf, ntiles, ssq_cols):
            """Pairwise-fold row sums: src_bf [P, ntiles, 512] bf16 (z^2) ->
            ssq_cols [P, ntiles] f32.  bf16 folds at DVE 2x down to w=64,
            f32 below (precision: bf16 partials cover <=8 terms)."""
            cur, w = src_bf, 512
            while w > 1:
                nw = w // 2
                dt = BF16 if nw > 32 else F32
                nxt = (ssq_cols if nw == 1 else
                       foldp.tile([P, ntiles * nw], dt, tag=f"f{nw}"))
                cv = cur.rearrange("p (t w) -> p t w", w=w)
                nv = nxt.rearrange("p (t w) -> p t w", w=nw)
                nc.vector.tensor_add(
                    out=nv, in0=cv[:, :, 0:nw], in1=cv[:, :, nw:w])
                cur, w = nxt, nw

        def stage_a(ch):
            s = st8[ch]
            yn, yt = s["yn"], s["yt"]
            # TensorE [1|g] matvec over transposed fp8 -> {sum z, d} per row.
            # Per-group [2,512] results are copied into one stacked [8,512]
            # SBUF tile (rows 2g:2g+2) so stage_b can transpose 4 groups at
            # a time.
            stk = stp.tile([P, 512], F32, tag="stk")
            for g in range(4):
                st_ps = psS.tile([2, 512], F32, tag="st")
                for c in range(4):
                    nc.tensor.matmul(
                        st_ps, og_t[:, c, :], yt[:, c, g * 512:(g + 1) * 512],
                        start=(c == 0), stop=(c == 3))
                dst = stk[32 * g:32 * g + 2, :]
                if g % 4 < CP_SCALAR:
                    nc.scalar.activation(out=dst, in_=st_ps, func=AF.Copy)
                else:
                    nc.vector.tensor_scalar(
                        out=dst, in0=st_ps, scalar1=1.0, scalar2=None,
                        op0=AL.mult)
            # z^2 row sums via square + pairwise tree folds (no accum_out --
            # the accumulate path costs ~1us/tile on HW).  ScalarE squares
            # SQ_SCAL tiles in one big activation, DVE squares the rest.
            ssq = stats.tile([P, CH], F32, tag="ssq")
            nb = CH - SQ_SCAL
            if nb:
                prodB = foldp.tile([P, nb * E], BF16, tag="prodB")
                nc.vector.tensor_mul(
                    out=prodB, in0=yn[:, 0:nb * E], in1=yn[:, 0:nb * E])
            if SQ_SCAL:
                prodA = foldp.tile([P, SQ_SCAL * E], BF16, tag="prodA")
                nc.scalar.activation(
                    out=prodA, in_=yn[:, nb * E:CH * E], func=AF.Square)
            if nb:
                tree_reduce(prodB, nb, ssq[:, 0:nb])
            if SQ_SCAL:
                tree_reduce(prodA, SQ_SCAL, ssq[:, nb:CH])
            s["stk"] = stk
            s["ssq"] = ssq

        def stage_b(ch):
            s = st8[ch]
            stk, ssq = s["stk"], s["ssq"]
            # 4 batched transposes [128,128] -> [128,128]: slice t4's output
            # cols 32g+s hold (sum z, d) of tile 4g+t4 (groups stacked at
            # partitions 32g in stk)
            stT_ps = psT.tile([P, 4 * P], F32, tag="stT")
            for t4 in range(4):
                nc.tensor.transpose(
                    stT_ps[:, t4 * P:(t4 + 1) * P],
                    stk[:, t4 * P:(t4 + 1) * P], ident)
            # de-permute into tile order: szd col 2t+s (t = 4g+t4) from
            # stT col 128*t4 + 32g + s -- per-g strided copies
            szd = stats.tile([P, 2 * CH], F32, tag="szd")
            stT_v = stT_ps.rearrange("p (t4 b) -> p t4 b", t4=4)
            for g in range(4):
                dstv = szd[:, 8 * g:8 * g + 8].rearrange(
                    "p (t4 s) -> p t4 s", s=2)
                nc.vector.tensor_scalar(
                    out=dstv, in0=stT_v[:, :, 32 * g:32 * g + 2],
                    scalar1=1.0, scalar2=None, op0=AL.mult)

            # ---- smalls: m, var, sigma^{+-1} via exp/ln, logits, q ----
            m_f = small.tile([P, CH], F32, tag="m")
            nc.vector.tensor_scalar(
                out=m_f, in0=szd[:, 0:2 * CH:2], scalar1=INV_E, scalar2=None,
                op0=AL.mult)
            ve = small.tile([P, CH], F32, tag="ve")
            nc.vector.tensor_scalar(
                out=ve, in0=ssq, scalar1=INV_E, scalar2=EPS,
                op0=AL.mult, op1=AL.add)
            m2 = small.tile([P, CH], F32, tag="m2")
            nc.vector.tensor_mul(out=m2, in0=m_f, in1=m_f)
            nc.vector.tensor_sub(out=ve, in0=ve, in1=m2)
            lnv = small.tile([P, CH], F32, tag="lnv")
            nc.scalar.activation(out=lnv, in_=ve, func=AF.Ln)
            isig = small.tile([P, CH], F32, tag="isig")
            nc.scalar.activation(out=isig, in_=lnv, func=AF.Exp, scale=-0.5)
            sig_bf = small.tile([P, CH], BF16, tag="sigbf")
            nc.scalar.activation(out=sig_bf, in_=lnv, func=AF.Exp, scale=0.5)
            # s = (d - m*S) * isig
            ms = small.tile([P, CH], F32, tag="ms")
            nc.vector.tensor_scalar(
                out=ms, in0=m_f, scalar1=sS_t, scalar2=None, op0=AL.mult)
            nc.vector.tensor_sub(out=ms, in0=szd[:, 1:2 * CH:2], in1=ms)
            nc.vector.tensor_mul(out=ms, in0=ms, in1=isig)
            exps = small.tile([P, CH], BF16, tag="exps")
            nc.scalar.activation(out=exps, in_=ms, func=AF.Exp)
            isig_bf = small.tile([P, CH], BF16, tag="isigbf")
            nc.vector.tensor_scalar(
                out=isig_bf, in0=isig, scalar1=1.0, scalar2=None, op0=AL.mult)
            q_bf = small.tile([P, CH], BF16, tag="qbf")
            nc.vector.tensor_mul(out=q_bf, in0=exps, in1=isig_bf)

            # write q into qf[ch%4]'s fixed block-diag column window:
            # col(t, h) = 130*t + 32*(ch%4) + h
            qfb = qf[ch % 4]
            base = 32 * (ch % 4)
            nc.vector.tensor_scalar(
                out=qfb[0:64, base::130], in0=q_bf[0:64, :],
                scalar1=1.0, scalar2=None, op0=AL.mult)
            nc.vector.tensor_scalar(
                out=qfb[64:128, base + 1::130], in0=q_bf[64:128, :],
                scalar1=1.0, scalar2=None, op0=AL.mult)
            # msig[:,2t] = m, msig[:,2t+1] = sigma (bf16)
            msig = small.tile([P, 2 * CH], BF16, tag="msig")
            mv = msig.rearrange("p (t two) -> p t two", two=2)
            nc.vector.tensor_scalar(
                out=mv[:, :, 0:1],
                in0=m_f.rearrange("p (t one) -> p t one", one=1),
                scalar1=1.0, scalar2=None, op0=AL.mult)
            nc.vector.tensor_scalar(
                out=mv[:, :, 1:2],
                in0=sig_bf.rearrange("p (t one) -> p t one", one=1),
                scalar1=1.0, scalar2=None, op0=AL.mult)
            s["msig"] = msig

        def stage_c(ch):
            s = st8[ch]
            yn, msig = s["yn"], s["msig"]
            qfb = qf[ch % 4]
            nck = ch // 4
            for t in range(CH):
                lhsT = qfb[:, t * P:(t + 1) * P]
                first = (ch % 4 == 0) and t == 0
                last = (ch % 4 == 3) and t == CH - 1
                nc.tensor.matmul(
                    agg_ps[nck], lhsT, yn[:, t * E:(t + 1) * E],
                    start=first, stop=last)
                nc.tensor.matmul(
                    rs_ps[nck], lhsT, msig[:, 2 * t:2 * t + 2],
                    start=first, stop=last)
            del st8[ch]

        # ---- final phase (split): head = normalize/transpose/W1g matmul/+x
        # (no ScalarE, so group 0's head can run mid-loop); gelu + store at
        # the very end (single activation-table switch).
        aggT = singles.tile([P, 4 * N], F32)  # [e_chunk(4) x nodes(256)]

        def final_head(ncx):
            rs_sb = fpool.tile([P, 2], F32, tag="rs")
            nc.vector.tensor_scalar(
                out=rs_sb, in0=rs_ps[ncx], scalar1=1.0, scalar2=None,
                op0=AL.mult)
            rinv = fpool.tile([P, 1], F32, tag="rinv")
            nc.vector.reciprocal(out=rinv, in_=rs_sb[:, 1:2])
            aggn = fpool.tile([P, E], F32, tag="aggn")
            nc.vector.tensor_scalar(
                out=aggn, in0=agg_ps[ncx], scalar1=rs_sb[:, 0:1],
                scalar2=rinv, op0=AL.subtract, op1=AL.mult)
            for c in range(4):
                tp = psT.tile([P, P], F32, tag="stT")
                nc.tensor.transpose(tp, aggn[:, c * P:(c + 1) * P], ident)
                nc.vector.tensor_scalar(
                    out=aggT[:, c * N + ncx * P: c * N + (ncx + 1) * P],
                    in0=tp, scalar1=1.0, scalar2=None, op0=AL.mult)
            fin = agg_ps[ncx]  # dead after aggn copy -> reuse the PSUM bank
            for c in range(4):
                nc.tensor.matmul(
                    fin, aggT[:, c * N + ncx * P: c * N + (ncx + 1) * P],
                    w1g_t[:, c, :], start=(c == 0), stop=False)
            nc.tensor.matmul(
                fin, ones_row[0:1, :], bb_t[0:1, :], start=False, stop=True)
            pre = fpool.tile([P, E], F32, tag=f"pre{ncx}")
            nc.vector.tensor_add(out=pre, in0=fin, in1=x_t[:, ncx, :])
            return pre

        pre_t = [None, None]
        for it in range(NCH + 2):
            if 1 <= it and it + 1 < NCH:
                stage_load(it + 1)
            if it == 3:
                load_final_params()
            if 1 <= it <= NCH:
                stage_b(it - 1)
            if it < NCH:
                stage_a(it)
            if it >= 2:
                stage_c(it - 2)
            if it == 6:
                pre_t[0] = final_head(0)
        pre_t[1] = final_head(1)

        gelu_fn = AF.Copy if SIM_NO_GELU else AF.Gelu_apprx_tanh
        for ncx in range(2):
            outt = fpool.tile([P, E], F32, tag="outt")
            nc.scalar.activation(out=outt, in_=pre_t[ncx], func=gelu_fn)
            nc.sync.dma_start(out=out_d[:, ncx, :], in_=outt)

    split_waits(nc)
    return nc


_NC_CACHE = {}


def make_in_maps(x, y, ln_gamma, ln_beta, W1, b1, W2, b2):
    x = np.asarray(x, np.float32)
    y = np.asarray(y, np.float32)
    ln_gamma = np.asarray(ln_gamma, np.float32)
    ln_beta = np.asarray(ln_beta, np.float32)
    W1 = np.asarray(W1, np.float32)
    b1 = np.asarray(b1, np.float32)
    W2 = np.asarray(W2, np.float32)

    # host-side precomputation (cheap, E-sized)
    w2y = W2[E:]
    v = W1 @ w2y                          # [E]
    g = (ln_gamma * v).astype(np.float32)  # [E]
    S = float(g.sum())
    w1g = (ln_gamma[:, None] * W1).astype(np.float32)      # [E, E]
    bb = (ln_beta @ W1 + b1).astype(np.float32)            # [E]

    og = np.empty((P, 4, 2), ml_dtypes.bfloat16)
    og[:, :, 0] = np.float32(1.0)
    og[:, :, 1] = g.reshape(4, P).T.astype(ml_dtypes.bfloat16)
    w1g_t = np.ascontiguousarray(
        w1g.reshape(4, P, E).transpose(1, 0, 2))           # [P, 4, E]
    bb_r = bb.reshape(1, E)
    sS = np.full((P, 1), S, np.float32)

    f8 = ml_dtypes.float8_e4m3fn
    y8 = y.reshape(B * L, K, E).astype(f8)                 # quantize once
    x_f = x.reshape(B * L, E)
    in_maps = []
    for i in range(NCORES):
        yc = y8[i * N:(i + 1) * N].reshape(R, E)           # [R, E] fp8
        # normal partition-major: [p, t*E+e] = yc[t*128+p, e]
        yn = np.ascontiguousarray(
            yc.reshape(T, P, E).transpose(1, 0, 2)).reshape(P, T * E)
        if not USE_CAST_DMA:
            yn = yn.astype(ml_dtypes.bfloat16)
        # host transpose: [p, c, r] = yc[r, c*128+p]
        yt = np.ascontiguousarray(yc.reshape(R, 4, P).transpose(2, 1, 0))
        xc = np.ascontiguousarray(
            x_f[i * N:(i + 1) * N].reshape(2, P, E).transpose(1, 0, 2))
        in_maps.append({
            "yn8": yn, "yt8": yt, "x": xc,
            "og": og, "w1g": w1g_t, "bb": bb_r, "sS": sS,
        })
    return in_maps


def kernel(x, y, ln_gamma, ln_beta, W1, b1, W2, b2, select_indegree_num=None,
           **kw):
    in_maps = make_in_maps(x, y, ln_gamma, ln_beta, W1, b1, W2, b2)
    if "nc" not in _NC_CACHE:
        _NC_CACHE["nc"] = build()
    nc = _NC_CACHE["nc"]

    res = run_bass_kernel_spmd(nc, in_maps, core_ids=list(range(NCORES)),
                               trace=bool(int(os.environ.get("KERNEL_TRACE", "0"))))
    _NC_CACHE["last_result"] = res
    # out [P, 2, E] node-major -> [N, E]
    out = np.concatenate(
        [np.asarray(r["out"]).transpose(1, 0, 2).reshape(N, E)
         for r in res.results], axis=0)
    return out.reshape(B, L, E)


# revision 23
# speedup vs baseline: 3.0313x; 1.0512x over previous
"""Trainium2 Bass kernel for nn_AttentionLayer (GNN message passing).

Math (per node n, K=64 neighbors, E=512), derived from the reference:
  - softmax over k is invariant to per-n shifts => prob depends only on
    s[n,k] = (d - m*S)/sigma, where d = y.g (g = gamma*(W1@w2y)), m/sigma
    the per-row LN stats, S = sum(g).  x path, b1, b2 cancel entirely.
  - a = ((sum_k q_k y_k) - (sum_k q_k m_k)) @ (diag(gamma) W1) + beta@W1 + b1
    with q_k = exp(s_k)/sigma_k / sumexp, sumexp = sum_k exp(s_k) = sum q~ sigma.

Device layout strategy (the baseline was DMA-packet-bound: xbar-transpose
DMAs at 256 B/packet + a DRAM stats bounce at 4 B/packet => ~167 GB/s
aggregate and 465 us).  Here ALL DMAs are large with per-partition
contiguous DRAM, permuted host-side:
  - yn8 [128, T*E] fp8:  [p, t*E+e] = y[128*t+p, e]  (normal, row-major tiles)
    loaded with a casting SWDGE DMA into bf16 SBUF chunks (halves HBM traffic)
  - yt8 [128, 4, R] fp8: [p, c, r] = y[r, 128*c+p]   (host-side transpose)
    feeds TensorE [1|g] matvecs => per-row {sum z, d} with NO on-chip transpose
  - z^2 stats via ScalarE Square+accum_out / DVE mult+accum split, from the
    bf16 normal-layout chunks.
  - stats [2, 512] PSUM blocks are relayouted to [row%128, tile] with thin
    PE transposes (no DRAM bounce).
  - sigma via exp(+-0.5*ln(var+eps)): Square/Ln/Exp share ONE activation
    table set (natural_log_exp_and_others) -> no table thrashing; one switch
    for the final Gelu.

Sharding: data-parallel over B*L across 8 cores, params replicated.
"""

import os
import numpy as np
import ml_dtypes
from contextlib import ExitStack

import concourse.bass as bass
import concourse.mybir as mybir
import concourse.tile as tile
from concourse.bass_utils import run_bass_kernel_spmd
from concourse.masks import make_identity

F32 = mybir.dt.float32
BF16 = mybir.dt.bfloat16
FP8 = mybir.dt.float8e4
AL = mybir.AluOpType
AF = mybir.ActivationFunctionType

B, L, K, E = 32, 64, 64, 512
NCORES = 8
N = B * L // NCORES          # 256 nodes per core
R = N * K                    # 16384 y-rows per core
P = 128                      # partitions
T = R // P                   # 128 tiles of [128, E] per core
CH = 16                      # tiles per chunk
NCH = T // CH                # 8 chunks
CHR = CH * P                 # 2048 rows per chunk
EPS = 1e-5
INV_E = 1.0 / E

# knobs
USE_CAST_DMA = bool(int(os.environ.get("KERNEL_CAST_DMA", "1")))
SQ_SCAL = int(os.environ.get("KERNEL_SQ_SCAL", "12"))  # z^2 tiles squared on ScalarE
STATS_DR = bool(int(os.environ.get("KERNEL_STATS_DR", "1")))  # DoubleRow fp8 stats MMs
CP_SCALAR = int(os.environ.get("KERNEL_CP_SCALAR", "4"))  # of 4 grp copies on ScalarE
SIM_NO_GELU = bool(int(os.environ.get("KERNEL_SIM_NO_GELU", "0")))  # CoreSim lacks Gelu


def split_waits(nc):
    """Workaround for this walrus build: most instruction structs encode only
    one sync-wait command, but Tile emits up to ~3 per instruction. Hoist all
    but the last wait onto same-engine NoOps spliced immediately before the
    instruction."""
    n_split = 0
    for f in nc.m.functions:
        for bb in f.blocks:
            insts = list(bb.instructions)
            out = []
            for inst in insts:
                si = inst.sync_info
                if si is not None and len(si.on_wait) > 1:
                    waits = list(si.on_wait)
                    for k, w in enumerate(waits[:-1]):
                        nop = mybir.InstNoOp(
                            name=f"{inst.name}-ws{k}", ins=[], outs=[])
                        nop.engine = inst.engine
                        nop.sync_info = mybir.SyncInfo(on_wait=[w],
                                                       on_update=[])
                        out.append(nop)
                        n_split += 1
                    inst.sync_info = mybir.SyncInfo(
                        on_wait=[waits[-1]], on_update=list(si.on_update))
                out.append(inst)
            bb.instructions = out
    return n_split


def build():
    nc = bass.Bass(trn_type="TRN2")

    yn_dt = FP8 if USE_CAST_DMA else BF16
    yn_d = nc.dram_tensor("yn8", [P, T * E], yn_dt, kind="ExternalInput")
    if STATS_DR:
        yt_d = nc.dram_tensor("yt8", [P, 2, 2, R], FP8, kind="ExternalInput")
        og_d = nc.dram_tensor("og", [P, 2, 2, 16], FP8, kind="ExternalInput")
    else:
        yt_d = nc.dram_tensor("yt8", [P, 4, R], FP8, kind="ExternalInput")
        og_d = nc.dram_tensor("og", [P, 4, 2], BF16, kind="ExternalInput")
    x_d = nc.dram_tensor("x", [P, 2, E], F32, kind="ExternalInput")
    w1g_d = nc.dram_tensor("w1g", [P, 4, E], BF16, kind="ExternalInput")
    bb_d = nc.dram_tensor("bb", [1, E], F32, kind="ExternalInput")
    sS_d = nc.dram_tensor("sS", [P, 1], F32, kind="ExternalInput")
    out_d = nc.dram_tensor("out", [P, 2, E], F32, kind="ExternalOutput")

    with tile.TileContext(nc) as tc, ExitStack() as ctx:
        singles = ctx.enter_context(tc.tile_pool(name="singles", bufs=1))
        ynp = ctx.enter_context(tc.tile_pool(name="ynp", bufs=3))
        ytp = ctx.enter_context(tc.tile_pool(name="ytp", bufs=2))
        stp = ctx.enter_context(tc.tile_pool(name="stp", bufs=3))
        stats = ctx.enter_context(tc.tile_pool(name="stats", bufs=3))
        foldp = ctx.enter_context(tc.tile_pool(name="foldp", bufs=2))
        small = ctx.enter_context(tc.tile_pool(name="small", bufs=3))
        fpool = ctx.enter_context(tc.tile_pool(name="fpool", bufs=2))
        psS = ctx.enter_context(tc.tile_pool(name="psS", bufs=2, space="PSUM"))
        psA = ctx.enter_context(tc.tile_pool(name="psA", bufs=1, space="PSUM"))
        psR = ctx.enter_context(tc.tile_pool(name="psR", bufs=1, space="PSUM"))
        psT = ctx.enter_context(tc.tile_pool(name="psT", bufs=2, space="PSUM"))

        # chunk-load stage, defined early so chunks 0/1 can be prefetched
        # ahead of the parameter loads (nothing blocks on params for a while)
        st8 = {}

        def stage_load(ch):
            yn = ynp.tile([P, CH * E], BF16, tag="yn")
            src = yn_d[:, ch * CH * E:(ch + 1) * CH * E]
            if USE_CAST_DMA:
                nc.gpsimd.dma_start(out=yn, in_=src)
            else:
                nc.sync.dma_start(out=yn, in_=src)
            if STATS_DR:
                yt = ytp.tile([P, 2, 2, CHR], FP8, tag="yt")
                nc.sync.dma_start(
                    out=yt, in_=yt_d[:, :, :, ch * CHR:(ch + 1) * CHR])
            else:
                yt = ytp.tile([P, 4, CHR], FP8, tag="yt")
                nc.sync.dma_start(
                    out=yt, in_=yt_d[:, :, ch * CHR:(ch + 1) * CHR])
            st8[ch] = {"yn": yn, "yt": yt}

        stage_load(0)
        stage_load(1)

        # ---- constants needed by the main loop ----
        if STATS_DR:
            og_t = singles.tile([P, 2, 2, 16], FP8)
            nc.sync.dma_start(out=og_t, in_=og_d[:, :, :, :])
        else:
            og_t = singles.tile([P, 4, 2], BF16)
            nc.sync.dma_start(out=og_t, in_=og_d[:, :, :])
        sS_t = singles.tile([P, 1], F32)
        nc.sync.dma_start(out=sS_t, in_=sS_d[:, :])
        ones_row = singles.tile([1, P], F32)
        nc.vector.memset(ones_row, 1.0)
        ident = singles.tile([P, P], F32)
        make_identity(nc, ident)
        eps_t = singles.tile([P, 1], F32)
        nc.vector.memset(eps_t, EPS)
        # final-phase params (loaded later, mid-loop, when SP has slack)
        w1g_t = singles.tile([P, 4, E], BF16)
        bb_t = singles.tile([1, E], F32)
        x_t = singles.tile([P, 2, E], F32)

        def load_final_params():
            nc.sync.dma_start(out=w1g_t, in_=w1g_d[:, :, :])
            nc.sync.dma_start(out=bb_t, in_=bb_d[:, :])
            nc.sync.dma_start(out=x_t, in_=x_d[:, :, :])

        # block-diag aggregation weights. Each buffer owns a FIXED disjoint
        # column window (win j covers local cols 32j..32j+31); anything else
        # stays zero forever, so a tile-slice lhsT never picks up stale q
        # from other chunks. Buffer j is reused by chunks j and j+4 (same
        # window; WAR tracked by Tile).
        qf = [singles.tile([P, CH * P], BF16, name=f"qf{i}") for i in range(4)]
        for i in range(4):
            nc.gpsimd.memset(qf[i], 0.0)

        # persistent PSUM accumulation targets (one per 128-node chunk)
        agg_ps = [psA.tile([P, E], F32, name=f"agg{i}") for i in range(2)]
        rs_ps = [psR.tile([P, 2], F32, name=f"rs{i}") for i in range(2)]

        # Software-pipelined emission: per iteration we emit
        #   Pf(ch+1): DMA prefetch          (issued 1 iter ahead)
        #   B(ch-1):  transposes/smalls/q   (consumes stats of prev chunk)
        #   A(ch):    stats MMs, z^2        (consumes prefetched loads)
        #   C(ch-2):  aggregation MMs       (consumes q of 2 chunks back)
        # so each engine's in-order queue only ever waits on work emitted a
        # full iteration earlier -> no head-of-line stalls.
        def tree_reduce(src_bf, ntiles, ssq_cols):
            """Pairwise-fold row sums: src_bf [P, ntiles, 512] bf16 (z^2) ->
            ssq_cols [P, ntiles] f32.  bf16 folds at DVE 2x down to w=64,
            f32 below (precision: bf16 partials cover <=8 terms)."""
            cur, w = src_bf, 512
            while w > 1:
                nw = w // 2
                dt = BF16 if nw > 32 else F32
                nxt = (ssq_cols if nw == 1 else
                       foldp.tile([P, ntiles * nw], dt, tag=f"f{nw}"))
                cv = cur.rearrange("p (t w) -> p t w", w=w)
                nv = nxt.rearrange("p (t w) -> p t w", w=nw)
                nc.vector.tensor_add(
                    out=nv, in0=cv[:, :, 0:nw], in1=cv[:, :, nw:w])
                cur, w = nxt, nw

        def stage_a(ch):
            s = st8[ch]
            yn, yt = s["yn"], s["yt"]
            # TensorE [1|g] matvec over transposed fp8 -> {sum z, d} per row.
            # Per-group [2,512] results are copied into one stacked [8,512]
            # SBUF tile (rows 2g:2g+2) so stage_b can transpose 4 groups at
            # a time.
            stk = stp.tile([P, 512], F32, tag="stk")
            for g in range(4):
                st_ps = psS.tile([2, 512], F32, tag="st")
                if STATS_DR:
                    for c in range(2):
                        nc.tensor.matmul(
                            st_ps, og_t[:, c, :, 0:2],
                            yt[:, c, :, g * 512:(g + 1) * 512],
                            start=(c == 0), stop=(c == 1),
                            perf_mode=mybir.MatmulPerfMode.DoubleRow)
                else:
                    for c in range(4):
                        nc.tensor.matmul(
                            st_ps, og_t[:, c, :],
                            yt[:, c, g * 512:(g + 1) * 512],
                            start=(c == 0), stop=(c == 3))
                dst = stk[32 * g:32 * g + 2, :]
                if g % 4 < CP_SCALAR:
                    nc.scalar.activation(out=dst, in_=st_ps, func=AF.Copy)
                else:
                    nc.vector.tensor_scalar(
                        out=dst, in0=st_ps, scalar1=1.0, scalar2=None,
                        op0=AL.mult)
            # z^2 row sums via square + pairwise tree folds (no accum_out --
            # the accumulate path costs ~1us/tile on HW).  ScalarE squares
            # SQ_SCAL tiles in one big activation, DVE squares the rest.
            ssq = stats.tile([P, CH], F32, tag="ssq")
            nb = CH - SQ_SCAL
            prod = foldp.tile([P, CH * E], BF16, tag="prod")
            if nb:
                nc.vector.tensor_mul(
                    out=prod[:, 0:nb * E], in0=yn[:, 0:nb * E],
                    in1=yn[:, 0:nb * E])
            if SQ_SCAL:
                nc.scalar.activation(
                    out=prod[:, nb * E:CH * E], in_=yn[:, nb * E:CH * E],
                    func=AF.Square)
            tree_reduce(prod, CH, ssq)
            s["stk"] = stk
            s["ssq"] = ssq

        def stage_b(ch):
            s = st8[ch]
            stk, ssq = s["stk"], s["ssq"]
            # 4 batched transposes [128,128] -> [128,128]: slice t4's output
            # cols 32g+s hold (sum z, d) of tile 4g+t4 (groups stacked at
            # partitions 32g in stk)
            stT_ps = psT.tile([P, 4 * P], F32, tag="stT")
            for t4 in range(4):
                nc.tensor.transpose(
                    stT_ps[:, t4 * P:(t4 + 1) * P],
                    stk[:, t4 * P:(t4 + 1) * P], ident)
            # de-permute into tile order: szd col 2t+s (t = 4g+t4) from
            # stT col 128*t4 + 32g + s -- per-g strided copies
            szd = stats.tile([P, 2 * CH], F32, tag="szd")
            stT_v = stT_ps.rearrange("p (t4 b) -> p t4 b", t4=4)
            for g in range(4):
                dstv = szd[:, 8 * g:8 * g + 8].rearrange(
                    "p (t4 s) -> p t4 s", s=2)
                nc.vector.tensor_scalar(
                    out=dstv, in0=stT_v[:, :, 32 * g:32 * g + 2],
                    scalar1=(1.0 / 16.0 if STATS_DR else 1.0), scalar2=None,
                    op0=AL.mult)

            # ---- smalls: m, var, sigma^{+-1} via exp/ln, logits, q ----
            m_f = small.tile([P, CH], F32, tag="m")
            nc.vector.tensor_scalar(
                out=m_f, in0=szd[:, 0:2 * CH:2], scalar1=INV_E, scalar2=None,
                op0=AL.mult)
            m2 = small.tile([P, CH], F32, tag="m2")
            nc.vector.tensor_mul(out=m2, in0=m_f, in1=m_f)
            ve = small.tile([P, CH], F32, tag="ve")
            nc.vector.scalar_tensor_tensor(
                out=ve, in0=ssq, scalar=INV_E, in1=m2,
                op0=AL.mult, op1=AL.subtract)
            lnv = small.tile([P, CH], F32, tag="lnv")
            nc.scalar.activation(out=lnv, in_=ve, func=AF.Ln, bias=eps_t)
            isig = small.tile([P, CH], F32, tag="isig")
            nc.scalar.activation(out=isig, in_=lnv, func=AF.Exp, scale=-0.5)
            sig_bf = small.tile([P, CH], BF16, tag="sigbf")
            nc.scalar.activation(out=sig_bf, in_=lnv, func=AF.Exp, scale=0.5)
            # s = (d - m*S) * isig
            ms = small.tile([P, CH], F32, tag="ms")
            nc.vector.tensor_scalar(
                out=ms, in0=m_f, scalar1=sS_t, scalar2=None, op0=AL.mult)
            nc.vector.tensor_sub(out=ms, in0=szd[:, 1:2 * CH:2], in1=ms)
            nc.vector.tensor_mul(out=ms, in0=ms, in1=isig)
            exps = small.tile([P, CH], BF16, tag="exps")
            nc.scalar.activation(out=exps, in_=ms, func=AF.Exp)
            isig_bf = small.tile([P, CH], BF16, tag="isigbf")
            nc.vector.tensor_scalar(
                out=isig_bf, in0=isig, scalar1=1.0, scalar2=None, op0=AL.mult)
            q_bf = small.tile([P, CH], BF16, tag="qbf")
            nc.vector.tensor_mul(out=q_bf, in0=exps, in1=isig_bf)

            # write q into qf[ch%4]'s fixed block-diag column window:
            # col(t, h) = 130*t + 32*(ch%4) + h
            qfb = qf[ch % 4]
            base = 32 * (ch % 4)
            nc.vector.tensor_scalar(
                out=qfb[0:64, base::130], in0=q_bf[0:64, :],
                scalar1=1.0, scalar2=None, op0=AL.mult)
            nc.vector.tensor_scalar(
                out=qfb[64:128, base + 1::130], in0=q_bf[64:128, :],
                scalar1=1.0, scalar2=None, op0=AL.mult)
            # msig[:,2t] = m, msig[:,2t+1] = sigma (bf16)
            msig = small.tile([P, 2 * CH], BF16, tag="msig")
            mv = msig.rearrange("p (t two) -> p t two", two=2)
            nc.vector.tensor_scalar(
                out=mv[:, :, 0:1],
                in0=m_f.rearrange("p (t one) -> p t one", one=1),
                scalar1=1.0, scalar2=None, op0=AL.mult)
            nc.vector.tensor_scalar(
                out=mv[:, :, 1:2],
                in0=sig_bf.rearrange("p (t one) -> p t one", one=1),
                scalar1=1.0, scalar2=None, op0=AL.mult)
            s["msig"] = msig

        def stage_c(ch):
            s = st8[ch]
            yn, msig = s["yn"], s["msig"]
            qfb = qf[ch % 4]
            nck = ch // 4
            for t in range(CH):
                lhsT = qfb[:, t * P:(t + 1) * P]
                first = (ch % 4 == 0) and t == 0
                last = (ch % 4 == 3) and t == CH - 1
                nc.tensor.matmul(
                    agg_ps[nck], lhsT, yn[:, t * E:(t + 1) * E],
                    start=first, stop=last)
                nc.tensor.matmul(
                    rs_ps[nck], lhsT, msig[:, 2 * t:2 * t + 2],
                    start=first, stop=last)
            del st8[ch]

        # ---- final phase (split): head = normalize/transpose/W1g matmul/+x
        # (no ScalarE, so group 0's head can run mid-loop); gelu + store at
        # the very end (single activation-table switch).
        aggT = singles.tile([P, 4 * N], BF16)  # [e_chunk(4) x nodes(256)]

        def final_head(ncx):
            rs_sb = fpool.tile([P, 2], F32, tag="rs")
            nc.vector.tensor_scalar(
                out=rs_sb, in0=rs_ps[ncx], scalar1=1.0, scalar2=None,
                op0=AL.mult)
            rinv = fpool.tile([P, 1], F32, tag="rinv")
            nc.vector.reciprocal(out=rinv, in_=rs_sb[:, 1:2])
            aggn = fpool.tile([P, E], F32, tag="aggn")
            nc.vector.tensor_scalar(
                out=aggn, in0=agg_ps[ncx], scalar1=rs_sb[:, 0:1],
                scalar2=rinv, op0=AL.subtract, op1=AL.mult)
            for c in range(4):
                tp = psT.tile([P, P], F32, tag="stT")
                nc.tensor.transpose(tp, aggn[:, c * P:(c + 1) * P], ident)
                nc.vector.tensor_scalar(
                    out=aggT[:, c * N + ncx * P: c * N + (ncx + 1) * P],
                    in0=tp, scalar1=1.0, scalar2=None, op0=AL.mult)
            fin = agg_ps[ncx]  # dead after aggn copy -> reuse the PSUM bank
            for c in range(4):
                nc.tensor.matmul(
                    fin, aggT[:, c * N + ncx * P: c * N + (ncx + 1) * P],
                    w1g_t[:, c, :], start=(c == 0), stop=False)
            nc.tensor.matmul(
                fin, ones_row[0:1, :], bb_t[0:1, :], start=False, stop=True)
            pre = fpool.tile([P, E], F32, tag=f"pre{ncx}")
            nc.vector.tensor_add(out=pre, in0=fin, in1=x_t[:, ncx, :])
            return pre

        pre_t = [None, None]
        for it in range(NCH + 2):
            if 1 <= it and it + 1 < NCH:
                stage_load(it + 1)
            if it == 3:
                load_final_params()
            if 1 <= it <= NCH:
                stage_b(it - 1)
            if it < NCH:
                stage_a(it)
            if it >= 2:
                stage_c(it - 2)
            if it == 6:
                pre_t[0] = final_head(0)
        pre_t[1] = final_head(1)

        gelu_fn = AF.Copy if SIM_NO_GELU else AF.Gelu_apprx_tanh
        for ncx in range(2):
            outt = fpool.tile([P, E], F32, tag="outt")
            nc.scalar.activation(out=outt, in_=pre_t[ncx], func=gelu_fn)
            nc.sync.dma_start(out=out_d[:, ncx, :], in_=outt)

    split_waits(nc)
    return nc


_NC_CACHE = {}


def make_in_maps(x, y, ln_gamma, ln_beta, W1, b1, W2, b2):
    x = np.asarray(x, np.float32)
    y = np.asarray(y, np.float32)
    ln_gamma = np.asarray(ln_gamma, np.float32)
    ln_beta = np.asarray(ln_beta, np.float32)
    W1 = np.asarray(W1, np.float32)
    b1 = np.asarray(b1, np.float32)
    W2 = np.asarray(W2, np.float32)

    # host-side precomputation (cheap, E-sized)
    w2y = W2[E:]
    v = W1 @ w2y                          # [E]
    g = (ln_gamma * v).astype(np.float32)  # [E]
    S = float(g.sum())
    w1g = (ln_gamma[:, None] * W1).astype(np.float32)      # [E, E]
    bb = (ln_beta @ W1 + b1).astype(np.float32)            # [E]

    f8 = ml_dtypes.float8_e4m3fn
    if STATS_DR:
        # [p, c2, ko, m(16-padded)]: 16*[1|g] (x16 dodges fp8 denormals for
        # small g; descaled by the szd copy), e = c2*256 + ko*128 + p
        og = np.zeros((P, 2, 2, 16), f8)
        og[:, :, :, 0] = np.float32(16.0)
        og[:, :, :, 1] = (16.0 * g).reshape(2, 2, P).transpose(2, 0, 1).astype(f8)
    else:
        og = np.empty((P, 4, 2), ml_dtypes.bfloat16)
        og[:, :, 0] = np.float32(1.0)
        og[:, :, 1] = g.reshape(4, P).T.astype(ml_dtypes.bfloat16)
    w1g_t = np.ascontiguousarray(
        w1g.reshape(4, P, E).transpose(1, 0, 2)).astype(
            ml_dtypes.bfloat16)                            # [P, 4, E]
    bb_r = bb.reshape(1, E)
    sS = np.full((P, 1), S, np.float32)

    y8 = y.reshape(B * L, K, E).astype(f8)                 # quantize once
    x_f = x.reshape(B * L, E)
    in_maps = []
    for i in range(NCORES):
        yc = y8[i * N:(i + 1) * N].reshape(R, E)           # [R, E] fp8
        # normal partition-major: [p, t*E+e] = yc[t*128+p, e]
        yn = np.ascontiguousarray(
            yc.reshape(T, P, E).transpose(1, 0, 2)).reshape(P, T * E)
        if not USE_CAST_DMA:
            yn = yn.astype(ml_dtypes.bfloat16)
        # host transpose: [p, c, r] = yc[r, c*128+p]; for DoubleRow the
        # c axis is split (c2, ko) = (c//2, c%2) -> [p, c2, ko, r]
        yt = np.ascontiguousarray(yc.reshape(R, 4, P).transpose(2, 1, 0))
        if STATS_DR:
            yt = yt.reshape(P, 2, 2, R)
        xc = np.ascontiguousarray(
            x_f[i * N:(i + 1) * N].reshape(2, P, E).transpose(1, 0, 2))
        in_maps.append({
            "yn8": yn, "yt8": yt, "x": xc,
            "og": og, "w1g": w1g_t, "bb": bb_r, "sS": sS,
        })
    return in_maps


def kernel(x, y, ln_gamma, ln_beta, W1, b1, W2, b2, select_indegree_num=None,
           **kw):
    in_maps = make_in_maps(x, y, ln_gamma, ln_beta, W1, b1, W2, b2)
    if "nc" not in _NC_CACHE:
        _NC_CACHE["nc"] = build()
    nc = _NC_CACHE["nc"]

    res = run_bass_kernel_spmd(nc, in_maps, core_ids=list(range(NCORES)),
                               trace=bool(int(os.environ.get("KERNEL_TRACE", "0"))))
    _NC_CACHE["last_result"] = res
    # out [P, 2, E] node-major -> [N, E]
    out = np.concatenate(
        [np.asarray(r["out"]).transpose(1, 0, 2).reshape(N, E)
         for r in res.results], axis=0)
    return out.reshape(B, L, E)


# revision 25
# speedup vs baseline: 3.5401x; 1.1678x over previous
"""Trainium2 Bass kernel for nn_AttentionLayer (GNN message passing).

Math (per node n, K=64 neighbors, E=512), derived from the reference:
  - softmax over k is invariant to per-n shifts => prob depends only on
    s[n,k] = (d - m*S)/sigma, where d = y.g (g = gamma*(W1@w2y)), m/sigma
    the per-row LN stats, S = sum(g).  x path, b1, b2 cancel entirely.
  - a = ((sum_k q_k y_k) - (sum_k q_k m_k)) @ (diag(gamma) W1) + beta@W1 + b1
    with q_k = exp(s_k)/sigma_k / sumexp, sumexp = sum_k exp(s_k) = sum q~ sigma.

Device layout strategy (the baseline was DMA-packet-bound: xbar-transpose
DMAs at 256 B/packet + a DRAM stats bounce at 4 B/packet => ~167 GB/s
aggregate and 465 us).  Here ALL DMAs are large with per-partition
contiguous DRAM, permuted host-side:
  - yn8 [128, T*E] fp8:  [p, t*E+e] = y[128*t+p, e]  (normal, row-major tiles)
    loaded with a casting SWDGE DMA into bf16 SBUF chunks (halves HBM traffic)
  - yt8 [128, 4, R] fp8: [p, c, r] = y[r, 128*c+p]   (host-side transpose)
    feeds TensorE [1|g] matvecs => per-row {sum z, d} with NO on-chip transpose
  - z^2 stats via ScalarE Square+accum_out / DVE mult+accum split, from the
    bf16 normal-layout chunks.
  - stats [2, 512] PSUM blocks are relayouted to [row%128, tile] with thin
    PE transposes (no DRAM bounce).
  - sigma via exp(+-0.5*ln(var+eps)): Square/Ln/Exp share ONE activation
    table set (natural_log_exp_and_others) -> no table thrashing; one switch
    for the final Gelu.

Sharding: data-parallel over B*L across 8 cores, params replicated.
"""

import os
import numpy as np
import ml_dtypes
from contextlib import ExitStack

import concourse.bass as bass
import concourse.mybir as mybir
import concourse.tile as tile
from concourse.bass_utils import run_bass_kernel_spmd
from concourse.masks import make_identity

F32 = mybir.dt.float32
BF16 = mybir.dt.bfloat16
FP8 = mybir.dt.float8e4
AL = mybir.AluOpType
AF = mybir.ActivationFunctionType

B, L, K, E = 32, 64, 64, 512
NCORES = 8
N = B * L // NCORES          # 256 nodes per core
R = N * K                    # 16384 y-rows per core
P = 128                      # partitions
T = R // P                   # 128 tiles of [128, E] per core
CH = 16                      # tiles per chunk
NCH = T // CH                # 8 chunks
CHR = CH * P                 # 2048 rows per chunk
EPS = 1e-5
INV_E = 1.0 / E

# knobs
USE_CAST_DMA = bool(int(os.environ.get("KERNEL_CAST_DMA", "1")))
SQ_SCAL = int(os.environ.get("KERNEL_SQ_SCAL", "13"))  # z^2 tiles squared on ScalarE
STATS_DR = bool(int(os.environ.get("KERNEL_STATS_DR", "1")))  # DoubleRow fp8 stats MMs
AGG_DR = bool(int(os.environ.get("KERNEL_AGG_DR", "1")))  # DoubleRow fp8 aggregation
CP_SCALAR = int(os.environ.get("KERNEL_CP_SCALAR", "4"))  # of 4 grp copies on ScalarE
SIM_NO_GELU = bool(int(os.environ.get("KERNEL_SIM_NO_GELU", "0")))  # CoreSim lacks Gelu


def split_waits(nc):
    """Workaround for this walrus build: most instruction structs encode only
    one sync-wait command, but Tile emits up to ~3 per instruction. Hoist all
    but the last wait onto same-engine NoOps spliced immediately before the
    instruction."""
    n_split = 0
    for f in nc.m.functions:
        for bb in f.blocks:
            insts = list(bb.instructions)
            out = []
            for inst in insts:
                si = inst.sync_info
                if si is not None and len(si.on_wait) > 1:
                    waits = list(si.on_wait)
                    for k, w in enumerate(waits[:-1]):
                        nop = mybir.InstNoOp(
                            name=f"{inst.name}-ws{k}", ins=[], outs=[])
                        nop.engine = inst.engine
                        nop.sync_info = mybir.SyncInfo(on_wait=[w],
                                                       on_update=[])
                        out.append(nop)
                        n_split += 1
                    inst.sync_info = mybir.SyncInfo(
                        on_wait=[waits[-1]], on_update=list(si.on_update))
                out.append(inst)
            bb.instructions = out
    return n_split


def build():
    nc = bass.Bass(trn_type="TRN2")

    yn_dt = FP8 if USE_CAST_DMA else BF16
    yn_d = nc.dram_tensor("yn8", [P, T * E], yn_dt, kind="ExternalInput")
    if STATS_DR:
        yt_d = nc.dram_tensor("yt8", [P, 2, 2, R], FP8, kind="ExternalInput")
        og_d = nc.dram_tensor("og", [P, 2, 2, 16], FP8, kind="ExternalInput")
    else:
        yt_d = nc.dram_tensor("yt8", [P, 4, R], FP8, kind="ExternalInput")
        og_d = nc.dram_tensor("og", [P, 4, 2], BF16, kind="ExternalInput")
    x_d = nc.dram_tensor("x", [P, 2, E], F32, kind="ExternalInput")
    w1g_d = nc.dram_tensor("w1g", [P, 4, E], BF16, kind="ExternalInput")
    bb_d = nc.dram_tensor("bb", [1, E], F32, kind="ExternalInput")
    sS_d = nc.dram_tensor("sS", [P, 1], F32, kind="ExternalInput")
    out_d = nc.dram_tensor("out", [P, 2, E], F32, kind="ExternalOutput")

    with tile.TileContext(nc) as tc, ExitStack() as ctx:
        singles = ctx.enter_context(tc.tile_pool(name="singles", bufs=1))
        ynp = ctx.enter_context(tc.tile_pool(name="ynp", bufs=3))
        ytp = ctx.enter_context(tc.tile_pool(name="ytp", bufs=2))
        stp = ctx.enter_context(tc.tile_pool(name="stp", bufs=3))
        stats = ctx.enter_context(tc.tile_pool(name="stats", bufs=3))
        foldp = ctx.enter_context(tc.tile_pool(name="foldp", bufs=2))
        small = ctx.enter_context(tc.tile_pool(name="small", bufs=3))
        fpool = ctx.enter_context(tc.tile_pool(name="fpool", bufs=2))
        psS = ctx.enter_context(tc.tile_pool(name="psS", bufs=2, space="PSUM"))
        psA = ctx.enter_context(tc.tile_pool(name="psA", bufs=1, space="PSUM"))
        psR = ctx.enter_context(tc.tile_pool(name="psR", bufs=1, space="PSUM"))
        psT = ctx.enter_context(tc.tile_pool(name="psT", bufs=2, space="PSUM"))

        # chunk-load stage, defined early so chunks 0/1 can be prefetched
        # ahead of the parameter loads (nothing blocks on params for a while)
        st8 = {}

        def stage_load(ch):
            yn = ynp.tile([P, CH * E], FP8 if AGG_DR else BF16, tag="yn")
            src = yn_d[:, ch * CH * E:(ch + 1) * CH * E]
            if USE_CAST_DMA and not AGG_DR:
                nc.gpsimd.dma_start(out=yn, in_=src)
            else:
                nc.sync.dma_start(out=yn, in_=src)
            if STATS_DR:
                yt = ytp.tile([P, 2, 2, CHR], FP8, tag="yt")
                nc.sync.dma_start(
                    out=yt, in_=yt_d[:, :, :, ch * CHR:(ch + 1) * CHR])
            else:
                yt = ytp.tile([P, 4, CHR], FP8, tag="yt")
                nc.sync.dma_start(
                    out=yt, in_=yt_d[:, :, ch * CHR:(ch + 1) * CHR])
            st8[ch] = {"yn": yn, "yt": yt}

        stage_load(0)
        stage_load(1)

        # ---- constants needed by the main loop ----
        if STATS_DR:
            og_t = singles.tile([P, 2, 2, 16], FP8)
            nc.sync.dma_start(out=og_t, in_=og_d[:, :, :, :])
        else:
            og_t = singles.tile([P, 4, 2], BF16)
            nc.sync.dma_start(out=og_t, in_=og_d[:, :, :])
        sS_t = singles.tile([P, 1], F32)
        nc.sync.dma_start(out=sS_t, in_=sS_d[:, :])
        ones_row = singles.tile([1, P], F32)
        nc.vector.memset(ones_row, 1.0)
        ident = singles.tile([P, P], F32)
        make_identity(nc, ident)
        eps_t = singles.tile([P, 1], F32)
        nc.vector.memset(eps_t, EPS)
        # final-phase params (loaded later, mid-loop, when SP has slack)
        w1g_t = singles.tile([P, 4, E], BF16)
        bb_t = singles.tile([1, E], F32)
        x_t = singles.tile([P, 2, E], F32)

        def load_final_params():
            nc.sync.dma_start(out=w1g_t, in_=w1g_d[:, :, :])
            nc.sync.dma_start(out=bb_t, in_=bb_d[:, :])
            nc.sync.dma_start(out=x_t, in_=x_d[:, :, :])

        # block-diag aggregation weights. Each buffer owns a FIXED disjoint
        # column window (win j covers local cols 32j..32j+31); anything else
        # stays zero forever, so a tile-slice lhsT never picks up stale q
        # from other chunks. Buffer j is reused by chunks j and j+4 (same
        # window; WAR tracked by Tile).
        if AGG_DR:
            qf = [singles.tile([P, 2, CH * P], FP8, name=f"qf{i}")
                  for i in range(4)]
        else:
            qf = [singles.tile([P, CH * P], BF16, name=f"qf{i}")
                  for i in range(4)]
        for i in range(4):
            nc.gpsimd.memset(qf[i], 0.0)

        # persistent PSUM accumulation targets (one per 128-node chunk)
        agg_ps = [psA.tile([P, E], F32, name=f"agg{i}") for i in range(2)]
        rs_ps = [psR.tile([P, 2], F32, name=f"rs{i}") for i in range(2)]

        # Software-pipelined emission: per iteration we emit
        #   Pf(ch+1): DMA prefetch          (issued 1 iter ahead)
        #   B(ch-1):  transposes/smalls/q   (consumes stats of prev chunk)
        #   A(ch):    stats MMs, z^2        (consumes prefetched loads)
        #   C(ch-2):  aggregation MMs       (consumes q of 2 chunks back)
        # so each engine's in-order queue only ever waits on work emitted a
        # full iteration earlier -> no head-of-line stalls.
        def tree_reduce(src_bf, ntiles, ssq_cols):
            """Pairwise-fold row sums: src_bf [P, ntiles, 512] bf16 (z^2) ->
            ssq_cols [P, ntiles] f32.  bf16 folds at DVE 2x down to w=64,
            f32 below (precision: bf16 partials cover <=8 terms)."""
            cur, w = src_bf, 512
            while w > 32:
                nw = w // 2
                dt = BF16 if nw > 32 else F32
                nxt = foldp.tile([P, ntiles * nw], dt, tag=f"f{nw}")
                cv = cur.rearrange("p (t w) -> p t w", w=w)
                nv = nxt.rearrange("p (t w) -> p t w", w=nw)
                nc.vector.tensor_add(
                    out=nv, in0=cv[:, :, 0:nw], in1=cv[:, :, nw:w])
                cur, w = nxt, nw
            nc.vector.tensor_reduce(
                out=ssq_cols, in_=cur.rearrange("p (t w) -> p t w", w=w),
                axis=mybir.AxisListType.X, op=AL.add)

        def stage_a(ch):
            s = st8[ch]
            yn, yt = s["yn"], s["yt"]
            # TensorE [1|g] matvec over transposed fp8 -> {sum z, d} per row.
            # Per-group [2,512] results are copied into one stacked [8,512]
            # SBUF tile (rows 2g:2g+2) so stage_b can transpose 4 groups at
            # a time.
            stk = stp.tile([P, 512], F32, tag="stk")
            for g in range(4):
                st_ps = psS.tile([2, 512], F32, tag="st")
                if STATS_DR:
                    for c in range(2):
                        nc.tensor.matmul(
                            st_ps, og_t[:, c, :, 0:2],
                            yt[:, c, :, g * 512:(g + 1) * 512],
                            start=(c == 0), stop=(c == 1),
                            perf_mode=mybir.MatmulPerfMode.DoubleRow)
                else:
                    for c in range(4):
                        nc.tensor.matmul(
                            st_ps, og_t[:, c, :],
                            yt[:, c, g * 512:(g + 1) * 512],
                            start=(c == 0), stop=(c == 3))
                dst = stk[32 * g:32 * g + 2, :]
                if g % 4 < CP_SCALAR:
                    nc.scalar.activation(out=dst, in_=st_ps, func=AF.Copy)
                else:
                    nc.vector.tensor_scalar(
                        out=dst, in0=st_ps, scalar1=1.0, scalar2=None,
                        op0=AL.mult)
            # z^2 row sums via square + pairwise tree folds (no accum_out --
            # the accumulate path costs ~1us/tile on HW).  ScalarE squares
            # SQ_SCAL tiles in one big activation, DVE squares the rest.
            ssq = stats.tile([P, CH], F32, tag="ssq")
            nb = CH - SQ_SCAL
            prod = foldp.tile([P, CH * E], BF16, tag="prod")
            if nb:
                nc.vector.tensor_mul(
                    out=prod[:, 0:nb * E], in0=yn[:, 0:nb * E],
                    in1=yn[:, 0:nb * E])
            if SQ_SCAL:
                nc.scalar.activation(
                    out=prod[:, nb * E:CH * E], in_=yn[:, nb * E:CH * E],
                    func=AF.Square)
            tree_reduce(prod, CH, ssq)
            s["stk"] = stk
            s["ssq"] = ssq

        def stage_b(ch):
            s = st8[ch]
            stk, ssq = s["stk"], s["ssq"]
            # 4 batched transposes [128,128] -> [128,128]: slice t4's output
            # cols 32g+s hold (sum z, d) of tile 4g+t4 (groups stacked at
            # partitions 32g in stk)
            stT_ps = psT.tile([P, 4 * P], F32, tag="stT")
            for t4 in range(4):
                nc.tensor.transpose(
                    stT_ps[:, t4 * P:(t4 + 1) * P],
                    stk[:, t4 * P:(t4 + 1) * P], ident)
            # de-permute into tile order: szd col 2t+s (t = 4g+t4) from
            # stT col 128*t4 + 32g + s -- per-g strided copies
            szd = stats.tile([P, 2 * CH], F32, tag="szd")
            stT_v = stT_ps.rearrange("p (t4 b) -> p t4 b", t4=4)
            for g in range(4):
                dstv = szd[:, 8 * g:8 * g + 8].rearrange(
                    "p (t4 s) -> p t4 s", s=2)
                nc.vector.tensor_scalar(
                    out=dstv, in0=stT_v[:, :, 32 * g:32 * g + 2],
                    scalar1=(1.0 / 16.0 if STATS_DR else 1.0), scalar2=None,
                    op0=AL.mult)

            # ---- smalls: m, var, sigma^{+-1} via exp/ln, logits, q ----
            m_f = small.tile([P, CH], F32, tag="m")
            nc.vector.tensor_scalar(
                out=m_f, in0=szd[:, 0:2 * CH:2], scalar1=INV_E, scalar2=None,
                op0=AL.mult)
            m2 = small.tile([P, CH], F32, tag="m2")
            nc.vector.tensor_mul(out=m2, in0=m_f, in1=m_f)
            ve = small.tile([P, CH], F32, tag="ve")
            nc.vector.scalar_tensor_tensor(
                out=ve, in0=ssq, scalar=INV_E, in1=m2,
                op0=AL.mult, op1=AL.subtract)
            lnv = small.tile([P, CH], F32, tag="lnv")
            nc.scalar.activation(out=lnv, in_=ve, func=AF.Ln, bias=eps_t)
            isig = small.tile([P, CH], F32, tag="isig")
            nc.scalar.activation(out=isig, in_=lnv, func=AF.Exp, scale=-0.5)
            sig_bf = small.tile([P, CH], BF16, tag="sigbf")
            nc.scalar.activation(out=sig_bf, in_=lnv, func=AF.Exp, scale=0.5)
            # s = (d - m*S) * isig
            ms = small.tile([P, CH], F32, tag="ms")
            nc.vector.tensor_scalar(
                out=ms, in0=m_f, scalar1=sS_t, scalar2=None, op0=AL.mult)
            nc.vector.tensor_sub(out=ms, in0=szd[:, 1:2 * CH:2], in1=ms)
            nc.vector.tensor_mul(out=ms, in0=ms, in1=isig)
            exps = small.tile([P, CH], BF16, tag="exps")
            nc.scalar.activation(out=exps, in_=ms, func=AF.Exp)
            isig_bf = small.tile([P, CH], BF16, tag="isigbf")
            nc.vector.tensor_scalar(
                out=isig_bf, in0=isig, scalar1=1.0, scalar2=None, op0=AL.mult)
            q_bf = small.tile([P, CH], BF16, tag="qbf")
            nc.vector.tensor_mul(out=q_bf, in0=exps, in1=isig_bf)

            qfb = qf[ch % 4]
            base = 32 * (ch % 4)
            if AGG_DR:
                # qf[ch%4] fp8 [Ki, Ko, cols]; pair u covers tiles (2u, 2u+1)
                # via Ko; node col (ko-plane) = 132u + base + 2ko + h
                for ko in range(2):
                    for h in range(2):
                        c0 = base + 2 * ko + h
                        nc.vector.tensor_scalar(
                            out=qfb[64 * h:64 * h + 64, ko,
                                    c0:c0 + 132 * (CH // 2):132],
                            in0=q_bf[64 * h:64 * h + 64, ko::2],
                            scalar1=1.0, scalar2=None, op0=AL.mult)
                # msig_dr fp8 [P, ko, (u, ms)]: m x16 (fp8 denormal dodge,
                # descaled in final rs read), sigma as-is
                msig = small.tile([P, 2, CH], FP8, tag="msig")
                for ko in range(2):
                    nc.vector.tensor_scalar(
                        out=msig[:, ko, 0::2], in0=m_f[:, ko::2],
                        scalar1=16.0, scalar2=None, op0=AL.mult)
                    nc.vector.tensor_scalar(
                        out=msig[:, ko, 1::2], in0=sig_bf[:, ko::2],
                        scalar1=1.0, scalar2=None, op0=AL.mult)
            else:
                # col(t, h) = 130*t + 32*(ch%4) + h
                nc.vector.tensor_scalar(
                    out=qfb[0:64, base::130], in0=q_bf[0:64, :],
                    scalar1=1.0, scalar2=None, op0=AL.mult)
                nc.vector.tensor_scalar(
                    out=qfb[64:128, base + 1::130], in0=q_bf[64:128, :],
                    scalar1=1.0, scalar2=None, op0=AL.mult)
                # msig[:,2t] = m, msig[:,2t+1] = sigma (bf16)
                msig = small.tile([P, 2 * CH], BF16, tag="msig")
                mv = msig.rearrange("p (t two) -> p t two", two=2)
                nc.vector.tensor_scalar(
                    out=mv[:, :, 0:1],
                    in0=m_f.rearrange("p (t one) -> p t one", one=1),
                    scalar1=1.0, scalar2=None, op0=AL.mult)
                nc.vector.tensor_scalar(
                    out=mv[:, :, 1:2],
                    in0=sig_bf.rearrange("p (t one) -> p t one", one=1),
                    scalar1=1.0, scalar2=None, op0=AL.mult)
            s["msig"] = msig

        def stage_c(ch):
            s = st8[ch]
            yn, msig = s["yn"], s["msig"]
            qfb = qf[ch % 4]
            nck = ch // 4
            if AGG_DR:
                for u in range(CH // 2):
                    lhsT = qfb[:, :, u * P:(u + 1) * P]
                    rhs = yn[:, 2 * u * E:(2 * u + 2) * E].rearrange(
                        "p (ko e) -> p ko e", ko=2)
                    first = (ch % 4 == 0) and u == 0
                    last = (ch % 4 == 3) and u == CH // 2 - 1
                    nc.tensor.matmul(
                        agg_ps[nck], lhsT, rhs, start=first, stop=last,
                        perf_mode=mybir.MatmulPerfMode.DoubleRow)
                    nc.tensor.matmul(
                        rs_ps[nck], lhsT, msig[:, :, 2 * u:2 * u + 2],
                        start=first, stop=last,
                        perf_mode=mybir.MatmulPerfMode.DoubleRow)
            else:
                for t in range(CH):
                    lhsT = qfb[:, t * P:(t + 1) * P]
                    first = (ch % 4 == 0) and t == 0
                    last = (ch % 4 == 3) and t == CH - 1
                    nc.tensor.matmul(
                        agg_ps[nck], lhsT, yn[:, t * E:(t + 1) * E],
                        start=first, stop=last)
                    nc.tensor.matmul(
                        rs_ps[nck], lhsT, msig[:, 2 * t:2 * t + 2],
                        start=first, stop=last)
            del st8[ch]

        # ---- final phase (split): head = normalize/transpose/W1g matmul/+x
        # (no ScalarE, so group 0's head can run mid-loop); gelu + store at
        # the very end (single activation-table switch).
        aggT = singles.tile([P, 4 * N], BF16)  # [e_chunk(4) x nodes(256)]

        def final_head(ncx):
            rs_sb = fpool.tile([P, 2], F32, tag="rs")
            nc.vector.tensor_scalar(
                out=rs_sb[:, 0:1], in0=rs_ps[ncx][:, 0:1],
                scalar1=(1.0 / 16.0 if AGG_DR else 1.0), scalar2=None,
                op0=AL.mult)
            nc.vector.tensor_scalar(
                out=rs_sb[:, 1:2], in0=rs_ps[ncx][:, 1:2],
                scalar1=1.0, scalar2=None, op0=AL.mult)
            rinv = fpool.tile([P, 1], F32, tag="rinv")
            nc.vector.reciprocal(out=rinv, in_=rs_sb[:, 1:2])
            aggn = fpool.tile([P, E], F32, tag="aggn")
            nc.vector.tensor_scalar(
                out=aggn, in0=agg_ps[ncx], scalar1=rs_sb[:, 0:1],
                scalar2=rinv, op0=AL.subtract, op1=AL.mult)
            for c in range(4):
                tp = psT.tile([P, P], F32, tag="stT")
                nc.tensor.transpose(tp, aggn[:, c * P:(c + 1) * P], ident)
                nc.vector.tensor_scalar(
                    out=aggT[:, c * N + ncx * P: c * N + (ncx + 1) * P],
                    in0=tp, scalar1=1.0, scalar2=None, op0=AL.mult)
            fin = agg_ps[ncx]  # dead after aggn copy -> reuse the PSUM bank
            for c in range(4):
                nc.tensor.matmul(
                    fin, aggT[:, c * N + ncx * P: c * N + (ncx + 1) * P],
                    w1g_t[:, c, :], start=(c == 0), stop=False)
            nc.tensor.matmul(
                fin, ones_row[0:1, :], bb_t[0:1, :], start=False, stop=True)
            pre = fpool.tile([P, E], F32, tag=f"pre{ncx}")
            nc.vector.tensor_add(out=pre, in0=fin, in1=x_t[:, ncx, :])
            return pre

        pre_t = [None, None]
        for it in range(NCH + 2):
            if 1 <= it and it + 1 < NCH:
                stage_load(it + 1)
            if it == 3:
                load_final_params()
            if 1 <= it <= NCH:
                stage_b(it - 1)
            if it < NCH:
                stage_a(it)
            if it >= 2:
                stage_c(it - 2)
            if it == 6:
                pre_t[0] = final_head(0)
        pre_t[1] = final_head(1)

        gelu_fn = AF.Copy if SIM_NO_GELU else AF.Gelu_apprx_tanh
        for ncx in range(2):
            outt = fpool.tile([P, E], F32, tag="outt")
            nc.scalar.activation(out=outt, in_=pre_t[ncx], func=gelu_fn)
            nc.sync.dma_start(out=out_d[:, ncx, :], in_=outt)

    split_waits(nc)
    return nc


_NC_CACHE = {}


def make_in_maps(x, y, ln_gamma, ln_beta, W1, b1, W2, b2):
    x = np.asarray(x, np.float32)
    y = np.asarray(y, np.float32)
    ln_gamma = np.asarray(ln_gamma, np.float32)
    ln_beta = np.asarray(ln_beta, np.float32)
    W1 = np.asarray(W1, np.float32)
    b1 = np.asarray(b1, np.float32)
    W2 = np.asarray(W2, np.float32)

    # host-side precomputation (cheap, E-sized)
    w2y = W2[E:]
    v = W1 @ w2y                          # [E]
    g = (ln_gamma * v).astype(np.float32)  # [E]
    S = float(g.sum())
    w1g = (ln_gamma[:, None] * W1).astype(np.float32)      # [E, E]
    bb = (ln_beta @ W1 + b1).astype(np.float32)            # [E]

    f8 = ml_dtypes.float8_e4m3fn
    if STATS_DR:
        # [p, c2, ko, m(16-padded)]: 16*[1|g] (x16 dodges fp8 denormals for
        # small g; descaled by the szd copy), e = c2*256 + ko*128 + p
        og = np.zeros((P, 2, 2, 16), f8)
        og[:, :, :, 0] = np.float32(16.0)
        og[:, :, :, 1] = (16.0 * g).reshape(2, 2, P).transpose(2, 0, 1).astype(f8)
    else:
        og = np.empty((P, 4, 2), ml_dtypes.bfloat16)
        og[:, :, 0] = np.float32(1.0)
        og[:, :, 1] = g.reshape(4, P).T.astype(ml_dtypes.bfloat16)
    w1g_t = np.ascontiguousarray(
        w1g.reshape(4, P, E).transpose(1, 0, 2)).astype(
            ml_dtypes.bfloat16)                            # [P, 4, E]
    bb_r = bb.reshape(1, E)
    sS = np.full((P, 1), S, np.float32)

    y8 = y.reshape(B * L, K, E).astype(f8)                 # quantize once
    x_f = x.reshape(B * L, E)
    in_maps = []
    for i in range(NCORES):
        yc = y8[i * N:(i + 1) * N].reshape(R, E)           # [R, E] fp8
        # normal partition-major: [p, t*E+e] = yc[t*128+p, e]
        yn = np.ascontiguousarray(
            yc.reshape(T, P, E).transpose(1, 0, 2)).reshape(P, T * E)
        if not USE_CAST_DMA:
            yn = yn.astype(ml_dtypes.bfloat16)
        # host transpose: [p, c, r] = yc[r, c*128+p]; for DoubleRow the
        # c axis is split (c2, ko) = (c//2, c%2) -> [p, c2, ko, r]
        yt = np.ascontiguousarray(yc.reshape(R, 4, P).transpose(2, 1, 0))
        if STATS_DR:
            yt = yt.reshape(P, 2, 2, R)
        xc = np.ascontiguousarray(
            x_f[i * N:(i + 1) * N].reshape(2, P, E).transpose(1, 0, 2))
        in_maps.append({
            "yn8": yn, "yt8": yt, "x": xc,
            "og": og, "w1g": w1g_t, "bb": bb_r, "sS": sS,
        })
    return in_maps


def kernel(x, y, ln_gamma, ln_beta, W1, b1, W2, b2, select_indegree_num=None,
           **kw):
    in_maps = make_in_maps(x, y, ln_gamma, ln_beta, W1, b1, W2, b2)
    if "nc" not in _NC_CACHE:
        _NC_CACHE["nc"] = build()
    nc = _NC_CACHE["nc"]

    res = run_bass_kernel_spmd(nc, in_maps, core_ids=list(range(NCORES)),
                               trace=bool(int(os.environ.get("KERNEL_TRACE", "0"))))
    _NC_CACHE["last_result"] = res
    # out [P, 2, E] node-major -> [N, E]
    out = np.concatenate(
        [np.asarray(r["out"]).transpose(1, 0, 2).reshape(N, E)
         for r in res.results], axis=0)
    return out.reshape(B, L, E)
